# revision 45
# baseline (speedup 1.0000x reference)
"""Trainium2 Bass kernel for nn_FEM_best (dual-attention fusion module).

Decomposition over 8 NeuronCores: core c = b*4 + r, b in {0,1}, r in:
  r=0: role S (computes E_s), pixel half 0   (phase-1: s-image stem, half 0)
  r=1: role S, half 1                        (phase-1: s-image stem, half 1)
  r=2: role Q (computes E_q), half 0         (phase-1: q-image stem, half 0)
  r=3: role Q, half 1                        (phase-1: q-image stem, half 1)
plus a channel-quarter of the final concat-conv on every core.

Single SPMD program; all per-core asymmetry is carried by input data
(weights, pre-padded image halves, and row-gather index tensors).
Cross-core exchange via AllGather collectives through DRAM bounce
tables + indirect row-gather DMAs.
"""
import sys, os
sys.path.insert(0, '/opt/trn_rl_repo')
import numpy as np

import concourse.bass as bass
import concourse.mybir as mybir
import concourse.bacc as bacc
import concourse.tile as tile
from concourse import bass_utils

F32 = mybir.dt.float32
F32R = mybir.dt.float32r
I32 = mybir.dt.int32
AF = mybir.ActivationFunctionType
ALU = mybir.AluOpType

B, CIN, H, W = 2, 256, 64, 64
N = H * W                 # 4096
C = 128                   # inter channels
CH = 64                   # C//2 (k/q projection channels)
CR = 8                    # gate bottleneck
HALF = N // 2             # 2048
PW = W + 2                # padded row width 66
NROW_IN = 34              # input rows per half (32 + 2 halo)
TC = 512                  # tile free size
NJ = HALF // TC           # 4 t-chunks per half
NU = N // 128             # 32 u-chunks
EXP_BIAS = -40.0
BF16 = mybir.dt.bfloat16

_cache = {}


def build_program():
    if 'nc' in _cache:
        return _cache['nc']
    nc = bacc.Bacc("TRN2", target_bir_lowering=False, debug=False, num_devices=8)

    def din(name, shape, dt=F32):
        return nc.dram_tensor(name, list(shape), dt, kind="ExternalInput").ap()

    def dout(name, shape, dt=F32):
        return nc.dram_tensor(name, list(shape), dt, kind="ExternalOutput").ap()

    i_img = din("img", [2, 128, NROW_IN * PW])          # padded input image half, 2 cin chunks
    i_wstem = din("wstem", [2, 9, 128, 128])            # (chunk, tap, ci, co) BN-folded
    i_bstem = din("bstem", [128, 1])
    i_wkp = din("wkp", [128, CH])                       # k-part projection (sign-folded)
    i_bkp = din("bkp", [CH, 1])
    i_wqp = din("wqp", [128, CH])
    i_bqp = din("bqp", [CH, 1])
    i_wv = din("wv", [128, 128])                        # cv_w^T [ci, c]
    i_bvrow = din("bvrow", [1, 128])                    # cv_b as a row
    i_wcc = din("wcc", [9, 128, 128])                   # cc conv, my role's cin half
    i_bcc = din("bcc", [128, 1])
    i_w1t = din("w1t", [128, CR])                       # gate MLP (mean-folded)
    i_b1 = din("b1", [CR, 1])
    i_w2t = din("w2t", [CR, 128])
    i_b2 = din("b2", [128, 1])
    ix_x1a = din("ix_x1a", [128, 1], I32)               # row-gather indices into AG2 table
    ix_x1b = din("ix_x1b", [128, 1], I32)
    ix_x2 = din("ix_x2", [128, 1], I32)
    ix_vp = din("ix_vp", [128, 1], I32)                 # partner v rows in ag2b table
    ix_d1 = din("ix_d1", [128, 1], I32)                 # fine-row indices into AG3 table
    ix_d2 = din("ix_d2", [128, 1], I32)
    ix_h1 = din("ix_h1", [128, 1], I32)                 # top-halo row gather
    ix_h2 = din("ix_h2", [128, 1], I32)                 # bottom-halo row gather
    ix_pp = din("ix_pp", [128, 1], I32)                 # partner cc-partial gather

    o_e = dout("o_e", [128, HALF])                      # E_{s|q} half (channel-major)
    o_cc = dout("o_cc", [128, HALF])                    # cpam pixel half (all channels)

    with tile.TileContext(nc) as tc:
        with tc.tile_pool(name="per", bufs=1) as per, \
             tc.tile_pool(name="dram", bufs=1, space="DRAM") as dram:
            # ---- persistent tiles ----
            x1a_lo = per.tile([128, HALF // 2], F32R)
            x1a_hi = per.tile([128, HALF // 2], F32R)
            x1b_lo = per.tile([128, HALF // 2], F32R)
            x1b_hi = per.tile([128, HALF // 2], F32R)
            x2_lo = per.tile([128, HALF // 2], F32R)
            x2_hi = per.tile([128, HALF // 2], F32R)
            vpm = per.tile([128, HALF], BF16)            # my v, pixel-major (u 0..15)
            vsec = per.tile([128, HALF], BF16)           # partner-half v (u 16..31)
            stem = per.tile([128, HALF], F32R)           # my stem output (= resid)
            gvec = per.tile([128, 1], F32)
            zslot = per.tile([128, NU], F32)
            zslot2 = per.tile([128, NU], F32)
            etile = per.tile([128, HALF], F32)
            rbc = per.tile([128, HALF], F32)
            bneg = per.tile([128, 1], F32)
            nc.gpsimd.memset(bneg[:], EXP_BIAS)
            ones_u = per.tile([128, 1], BF16)
            nc.gpsimd.memset(ones_u[:], 1.0)

            # kq exchange split into column halves: part 1 (each section's
            # first 1024 pixels) triggers after stem chunk 1, so scores on
            # the available (u, t) sub-blocks overlap part 2's transfer.
            ag2a1_in = dram.tile([128, 1024], F32)
            ag2a1_out = dram.tile([512, 1024], F32)
            ag2a2_in = dram.tile([128, 1024], F32)
            ag2a2_out = dram.tile([512, 1024], F32)
            ag2b_in = dram.tile([128, 2048], BF16)       # my v (pixel-major), bf16
            ag2b_out = dram.tile([256, 2048], BF16)      # 2-rank: [half0 v; half1 v]
            ag3_in = dram.tile([128, 32], F32)
            ag3_out = dram.tile([512, 32], F32)
            agh_in = dram.tile([128, 192], F32)          # halo: top row | bottom row | zeros
            agh_out = dram.tile([256, 192], F32)         # 2-rank same-role pair
            agp_in = dram.tile([128, 2048], BF16)        # cc partial sums, bf16
            agp_out = dram.tile([256, 2048], BF16)       # 2-rank cross-role pair
            dsc = dram.tile([16, 128], F32)             # transpose scratch for D row

            # ================= Phase A: stem + projections =================
            with nc.named_scope("pA_stem"), \
                 tc.tile_pool(name="pha", bufs=1) as pha, \
                 tc.tile_pool(name="psA", bufs=2, space="PSUM") as psA:
                # weights first so the first stem matmul isn't DMA-queued
                wstem = pha.tile([128, 2, 9, 128], F32R)
                nc.sync.dma_start(wstem[:], i_wstem[:].rearrange("a t p c -> p a t c").bitcast(F32R))
                bstem = pha.tile([128, 1], F32)
                nc.sync.dma_start(bstem[:], i_bstem[:])
                wkp = pha.tile([128, CH], F32R)
                nc.sync.dma_start(wkp[:], i_wkp[:].bitcast(F32R))
                bkp = pha.tile([CH, 1], F32)
                nc.sync.dma_start(bkp[:], i_bkp[:])
                wqp = pha.tile([128, CH], F32R)
                nc.sync.dma_start(wqp[:], i_wqp[:].bitcast(F32R))
                bqp = pha.tile([CH, 1], F32)
                nc.sync.dma_start(bqp[:], i_bqp[:])
                img = pha.tile([128, 2, NROW_IN * PW + 2], F32R)
                isrc = i_img[:].rearrange("a p x -> p a x").bitcast(F32R)
                for r0, r1 in ((0, 10), (10, 18), (18, 26), (26, 34)):
                    nc.sync.dma_start(img[:, :, r0 * PW:r1 * PW],
                                      isrc[:, :, r0 * PW:r1 * PW])

                kpart = pha.tile([CH, HALF], F32R)
                qpart = pha.tile([CH, HALF], F32R)
                for pc in range(NJ):                    # 4 pixel chunks of 512 (8 img rows)
                    sl = slice(pc * TC, (pc + 1) * TC)
                    ag_in = ag2a1_in if pc < 2 else ag2a2_in
                    asl = slice((pc % 2) * TC, (pc % 2) * TC + TC)
                    ps = psA.tile([128, TC], F32)
                    first = True
                    for ch in range(2):
                        for t in range(9):
                            dy, dx = t // 3 - 1, t % 3 - 1
                            off = (8 * pc + dy + 1) * PW + (dx + 1)
                            rhs = img[:, ch, off:off + 8 * PW].rearrange(
                                "p (r w) -> p r w", r=8)[:, :, 0:64]
                            nc.tensor.matmul(ps[:], wstem[:, ch, t, :], rhs,
                                             start=first, stop=(ch == 1 and t == 8))
                            first = False
                    nc.scalar.activation(stem[:, sl], ps[:], AF.Relu, bias=bstem[:])
                    # fused per-chunk projections + collective payload
                    psk = psA.tile([CH, TC], F32)
                    nc.tensor.matmul(psk[:], wkp[:], stem[:, sl], start=True, stop=True)
                    nc.scalar.activation(kpart[:, sl], psk[:], AF.Identity, bias=bkp[:])
                    psq = psA.tile([CH, TC], F32)
                    nc.tensor.matmul(psq[:], wqp[:], stem[:, sl], start=True, stop=True)
                    nc.scalar.activation(qpart[:, sl], psq[:], AF.Identity, bias=bqp[:])
                    nc.sync.dma_start(ag_in[0:64, asl], kpart[:, sl].bitcast(F32))
                    nc.sync.dma_start(ag_in[64:128, asl], qpart[:, sl].bitcast(F32))
                    if pc == 1:
                        with nc.named_scope("ag2a1"):
                            nc.gpsimd.collective_compute(
                                "AllGather", ALU.bypass,
                                replica_groups=[[0, 1, 2, 3], [4, 5, 6, 7]],
                                ins=[ag2a1_in.opt()], outs=[ag2a1_out.opt()],
                            )

                with nc.named_scope("ag2a2"):
                    nc.gpsimd.collective_compute(
                        "AllGather", ALU.bypass,
                        replica_groups=[[0, 1, 2, 3], [4, 5, 6, 7]],
                        ins=[ag2a2_in.opt()], outs=[ag2a2_out.opt()],
                    )

                # v (pixel-major, bf16) computed after the kq collective is
                # triggered; its 2-rank exchange rides behind ag2a on the
                # collective engine while scores run.
                wv = pha.tile([128, 128], F32)
                nc.sync.dma_start(wv[:], i_wv[:])
                bvrow = pha.tile([1, 128], F32)
                nc.sync.dma_start(bvrow[:], i_bvrow[:])
                bvbc = pha.tile([128, 128], F32)
                nc.gpsimd.partition_broadcast(bvbc[:], bvrow[:])
                for uc in range(16):
                    psv = psA.tile([128, 128], F32)
                    nc.tensor.matmul(psv[:], stem[:, uc * 128:(uc + 1) * 128].bitcast(F32),
                                     wv[:], start=True, stop=True)
                    nc.vector.tensor_tensor(vpm[:, uc * 128:(uc + 1) * 128],
                                            psv[:], bvbc[:], ALU.add)
                nc.sync.dma_start(ag2b_in[:], vpm[:])

                with nc.named_scope("ag2b"):
                    nc.gpsimd.collective_compute(
                        "AllGather", ALU.bypass,
                        replica_groups=[[0, 1], [2, 3], [4, 5], [6, 7]],
                        ins=[ag2b_in.opt()], outs=[ag2b_out.opt()],
                    )

            # ================= Phase B: gathers =================
            with nc.named_scope("pB_gather"), tc.tile_pool(name="phb", bufs=1) as phb:
                def gather(table, dst, idx_dram, cast=True):
                    idxt = phb.tile([128, 1], I32, name=f"idx_{dst.tensor.name}")
                    nc.sync.dma_start(idxt[:], idx_dram[:])
                    srcv = table[:].bitcast(F32R) if cast else table[:]
                    nc.gpsimd.indirect_dma_start(
                        out=dst[:], out_offset=None, in_=srcv,
                        in_offset=bass.IndirectOffsetOnAxis(ap=idxt[:, :1], axis=0))

                gather(ag2a1_out, x1a_lo, ix_x1a)
                gather(ag2a1_out, x2_lo, ix_x2)
                gather(ag2a1_out, x1b_lo, ix_x1b)
                gather(ag2a2_out, x1a_hi, ix_x1a)
                gather(ag2a2_out, x2_hi, ix_x2)
                gather(ag2a2_out, x1b_hi, ix_x1b)
                gather(ag2b_out, vsec, ix_vp, cast=False)

            # prefetch concat-conv weights during attention
            wcc = per.tile([128, 9, 128], F32R)
            nc.sync.dma_start(wcc[:], i_wcc[:].rearrange("t p c -> p t c").bitcast(F32R))
            bcc = per.tile([128, 1], F32)
            nc.sync.dma_start(bcc[:], i_bcc[:])

            # ================= Phase C: attention =================
            with nc.named_scope("pC_attn"), tc.tile_pool(name="phc", bufs=3) as phc, \
                 tc.tile_pool(name="psS", bufs=3, space="PSUM") as psS, \
                 tc.tile_pool(name="psO", bufs=1, space="PSUM") as psO:
                pv_ps = []
                for j in range(NJ):
                    pv_ps.append(psO.tile([128, TC], F32, name=f"pvps{j}"))
                LAG = 6
                HH = HALF // 2
                x1seg = {0: x1a_lo, 1: x1a_hi, 2: x1b_lo, 3: x1b_hi}
                # score sub-steps (u, t-half) ordered by collective availability:
                # phase 1 needs only ag2a1 (x1 lo segments + x2_lo)
                sched = [(u, 0) for u in range(8)] + [(u, 0) for u in range(16, 24)]
                for i in range(8):
                    sched += [(8 + i, 0), (i, 1), (24 + i, 0), (16 + i, 1)]
                for i in range(8):
                    sched += [(8 + i, 1), (24 + i, 1)]
                assert len(sched) == 2 * NU

                pts = {}
                deferred = []

                def emit_pv(uu, th):
                    vt = vpm if uu < 16 else vsec
                    uslv = slice((uu % 16) * 128, (uu % 16) * 128 + 128)
                    ptv = pts[uu]
                    for j in (2 * th, 2 * th + 1):
                        tsl = slice(j * TC, (j + 1) * TC)
                        nc.tensor.matmul(pv_ps[j][:], vt[:, uslv], ptv[:, tsl],
                                         start=(uu == 0), stop=(uu == NU - 1))

                for k, (u, th) in enumerate(sched):
                    x1t = x1seg[(u // 16) * 2 + (0 if u % 16 < 8 else 1)]
                    usl = slice((u % 8) * 128, (u % 8) * 128 + 128)
                    x2t = x2_lo if th == 0 else x2_hi
                    pabs = phc.tile([128, HH], F32, name="pabs", bufs=4)
                    sps = psS.tile([128, 2 * TC], F32, name="sps", bufs=2)
                    for jj in range(2):
                        nc.tensor.matmul(
                            sps[:, jj * TC:(jj + 1) * TC], x1t[:, usl],
                            x2t[:, jj * TC:(jj + 1) * TC],
                            start=True, stop=True)
                    nc.vector.tensor_scalar(pabs[:].bitcast(I32),
                                            sps[:].bitcast(I32),
                                            0x7FFFFFFF, None, ALU.bitwise_and)
                    if u not in pts:
                        pts[u] = phc.tile([128, HALF], BF16, name="p_t", bufs=24)
                    zdst = zslot if th == 0 else zslot2
                    nc.scalar.activation(pts[u][:, th * HH:(th + 1) * HH], pabs[:],
                                         AF.Exp, bias=bneg[:],
                                         accum_out=zdst[:, u:u + 1])
                    # PV emission: local-v sub-steps trail by LAG; partner-v
                    # ones wait until the v exchange has landed (~pos 48)
                    if k >= LAG:
                        uu, tth = sched[k - LAG]
                        if uu < 16:
                            emit_pv(uu, tth)
                        else:
                            deferred.append((uu, tth))
                    if k >= 48:
                        for _ in range(2):
                            if deferred:
                                emit_pv(*deferred.pop(0))
                # drain: remaining LAG window, then leftover partner-v PVs
                for k in range(2 * NU - LAG, 2 * NU):
                    uu, tth = sched[k]
                    if uu < 16:
                        emit_pv(uu, tth)
                    else:
                        deferred.append((uu, tth))
                while deferred:
                    emit_pv(*deferred.pop(0))

                # gate: sigmoid(w2 @ relu(w1 @ mean(v) + b1) + b2), mean folded in w1
                w1t = phc.tile([128, CR], F32, name="w1t")
                nc.sync.dma_start(w1t[:], i_w1t[:])
                b1 = phc.tile([CR, 1], F32, name="b1")
                nc.sync.dma_start(b1[:], i_b1[:])
                w2t = phc.tile([CR, 128], F32, name="w2t")
                nc.sync.dma_start(w2t[:], i_w2t[:])
                b2 = phc.tile([128, 1], F32, name="b2")
                nc.sync.dma_start(b2[:], i_b2[:])
                psum_v = psS.tile([128, 1], F32, name="psum_v", tag="sps", bufs=2)
                for half, vt in ((0, vpm), (1, vsec)):
                    for uc in range(16):
                        nc.tensor.matmul(psum_v[:],
                                         vt[:, uc * 128:(uc + 1) * 128],
                                         ones_u[:], start=(half == 0 and uc == 0),
                                         stop=(half == 1 and uc == 15))
                vsum = phc.tile([128, 1], F32, name="vsum")
                nc.scalar.copy(vsum[:], psum_v[:])
                psh = psS.tile([CR, 1], F32, name="psh", tag="sps", bufs=2)
                nc.tensor.matmul(psh[:], w1t[:], vsum[:], start=True, stop=True)
                hgate = phc.tile([CR, 1], F32, name="hgate")
                nc.scalar.activation(hgate[:], psh[:], AF.Relu, bias=b1[:])
                psg = psS.tile([128, 1], F32, name="psg", tag="sps", bufs=2)
                nc.tensor.matmul(psg[:], w2t[:], hgate[:], start=True, stop=True)
                nc.scalar.activation(gvec[:], psg[:], AF.Sigmoid, bias=b2[:])

                zsum = phc.tile([128, NU], F32, name="zsum")
                nc.vector.tensor_tensor(zsum[:], zslot[:], zslot2[:], ALU.add)
                nc.sync.dma_start(ag3_in[:], zsum[:])

                with nc.named_scope("ag3"):
                    nc.gpsimd.collective_compute(
                        "AllGather", ALU.bypass,
                        replica_groups=[[0, 1, 2, 3], [4, 5, 6, 7]],
                        ins=[ag3_in.opt()], outs=[ag3_out.opt()],
                    )

                # D = sum of the two other-role partials, my half (fine rows of 16)
                with nc.named_scope("pD_efin"):
                    ag3f = ag3_out[:].rearrange("r (g w) -> (r g) w", w=16)
                    w1g = phc.tile([128, 16], F32, name="w1g")
                    w2g = phc.tile([128, 16], F32, name="w2g")
                    for dst, ixd in ((w1g, ix_d1), (w2g, ix_d2)):
                        idxt = phc.tile([128, 1], I32, name=f"ixd_{dst.tensor.name}")
                        nc.sync.dma_start(idxt[:], ixd[:])
                        nc.gpsimd.indirect_dma_start(
                            out=dst[:], out_offset=None, in_=ag3f,
                            in_offset=bass.IndirectOffsetOnAxis(ap=idxt[:, :1], axis=0))
                    dmine = phc.tile([128, 16], F32, name="dmine")
                    nc.vector.tensor_tensor(dmine[:], w1g[:], w2g[:], ALU.add)
                    rrec = phc.tile([128, 16], F32, name="rrec")
                    nc.vector.reciprocal(rrec[:], dmine[:])
                    nc.sync.dma_start(dsc[:].rearrange("c p -> p c"), rrec[:])
                    drow = phc.tile([1, HALF], F32, name="drow")
                    nc.sync.dma_start(drow[:], dsc[:].rearrange("c p -> (c p)").unsqueeze(0))
                    nc.gpsimd.partition_broadcast(rbc[:], drow[:])

                    # E = (PV * g) * R + resid; edge chunks first so the halo
                    # exchange can start while the interior finishes
                    halo = phc.tile([128, 192], F32, name="halo")
                    nc.vector.memset(halo[:, 128:192], 0.0)
                    for j in (0, 3, 1, 2):
                        tsl = slice(j * TC, (j + 1) * TC)
                        nc.vector.scalar_tensor_tensor(etile[:, tsl], pv_ps[j][:],
                                                       gvec[:], rbc[:, tsl],
                                                       ALU.mult, ALU.mult)
                        nc.vector.tensor_tensor(etile[:, tsl], etile[:, tsl],
                                                stem[:, tsl].bitcast(F32), ALU.add)
                        if j == 0:
                            nc.vector.tensor_copy(halo[:, 0:64], etile[:, 0:64])
                        elif j == 3:
                            nc.vector.tensor_copy(halo[:, 64:128],
                                                  etile[:, HALF - 64:HALF])
                            nc.sync.dma_start(agh_in[:], halo[:])
                    for j in range(NJ):
                        tsl = slice(j * TC, (j + 1) * TC)
                        nc.sync.dma_start(o_e[:, tsl], etile[:, tsl])

            with nc.named_scope("agh"):
                nc.gpsimd.collective_compute(
                    "AllGather", ALU.bypass,
                    replica_groups=[[0, 1], [2, 3], [4, 5], [6, 7]],
                    ins=[agh_in.opt()], outs=[agh_out.opt()],
                )

            # ============ Phase E: concat conv via per-role partials ============
            with nc.named_scope("pE_cc"), tc.tile_pool(name="phe", bufs=1) as phe, \
                 tc.tile_pool(name="psE", bufs=2, space="PSUM") as psE:
                EPW = NROW_IN * PW + 2                   # 34 padded rows of 66 (+slack)
                epadS = phe.tile([128, EPW], F32R)       # my E half, padded
                zsrc = phe.tile([128, PW], F32R)
                nc.vector.memset(zsrc[:].bitcast(F32), 0.0)
                # interior rows 1..32 <- my E (restride 64 -> 66)
                nc.sync.dma_start(
                    epadS[:, PW + 1:33 * PW + 1].rearrange("p (r w) -> p r w", w=PW)[:, :, 0:64],
                    etile[:].rearrange("p (r w) -> p r w", w=64).bitcast(F32R))
                # halo rows 0 and 33 via indexed gather from the halo table
                aghf = agh_out[:].rearrange("r (g w) -> (r g) w", w=64).bitcast(F32R)
                for ixd, row in ((ix_h1, 0), (ix_h2, 33)):
                    idxt = phe.tile([128, 1], I32, name=f"ixh_{row}")
                    nc.sync.dma_start(idxt[:], ixd[:])
                    nc.gpsimd.indirect_dma_start(
                        out=epadS[:, row * PW + 1:row * PW + 65], out_offset=None,
                        in_=aghf,
                        in_offset=bass.IndirectOffsetOnAxis(ap=idxt[:, :1], axis=0))
                # zero borders: left/right columns of all 34 rows
                lcol = epadS[:, 0:34 * PW].rearrange("p (r w) -> p r w", w=PW)[:, :, 0:1]
                nc.sync.dma_start(lcol, zsrc[:, 0:34].unsqueeze(2))
                rcol = epadS[:, 65:34 * PW - 1].rearrange("p (r w) -> p r w", w=PW)[:, :, 0:1]
                nc.sync.dma_start(rcol, zsrc[:, 0:33].unsqueeze(2))
                nc.sync.dma_start(epadS[:, 34 * PW - 1:34 * PW + 1], zsrc[:, 0:2])

                pcc = phe.tile([128, HALF], BF16)
                for pc in (1, 2, 0, 3):
                    ps = psE.tile([128, TC], F32)
                    for t in range(9):
                        dy, dx = t // 3 - 1, t % 3 - 1
                        off = (8 * pc + dy + 1) * PW + (dx + 1)
                        rhs = epadS[:, off:off + 8 * PW].rearrange(
                            "p (r w) -> p r w", r=8)[:, :, 0:64]
                        nc.tensor.matmul(ps[:], wcc[:, t, :], rhs,
                                         start=(t == 0), stop=(t == 8))
                    nc.scalar.copy(pcc[:, pc * TC:(pc + 1) * TC], ps[:])
                    nc.sync.dma_start(agp_in[:, pc * TC:(pc + 1) * TC],
                                      pcc[:, pc * TC:(pc + 1) * TC])

                with nc.named_scope("agp"):
                    nc.gpsimd.collective_compute(
                        "AllGather", ALU.bypass,
                        replica_groups=[[0, 2], [1, 3], [4, 6], [5, 7]],
                        ins=[agp_in.opt()], outs=[agp_out.opt()],
                    )
                ppart = phe.tile([128, HALF], BF16)
                idxp = phe.tile([128, 1], I32)
                nc.sync.dma_start(idxp[:], ix_pp[:])
                nc.gpsimd.indirect_dma_start(
                    out=ppart[:], out_offset=None, in_=agp_out[:],
                    in_offset=bass.IndirectOffsetOnAxis(ap=idxp[:, :1], axis=0))
                csum = phe.tile([128, HALF], BF16)
                nc.vector.tensor_tensor(csum[:], pcc[:], ppart[:], ALU.add)
                ccout = phe.tile([128, HALF], F32)
                nc.scalar.activation(ccout[:], csum[:], AF.Relu, bias=bcc[:])
                nc.sync.dma_start(o_cc[:], ccout[:])

    nc.compile()
    _cache['nc'] = nc
    return nc


# ====================== host-side preparation ======================

def _prep_inputs(inp):
    """Build the 8 per-core input dicts from the full problem inputs."""
    f32 = np.float32
    g = {k: np.asarray(v, f32) for k, v in inp.items()}
    eps = 1e-5

    def fold_stem(w, b, gam, be, m, v):
        s = gam / np.sqrt(v + eps)
        w_eff = w * s[:, None, None, None]                     # [co, cin, 3, 3]
        b_eff = (b - m) * s + be
        # [2, 9, 128, 128] : (cin chunk, tap, ci, co)
        wt = np.zeros((2, 9, 128, 128), f32)
        for ch in range(2):
            for t in range(9):
                wt[ch, t] = w_eff[:, ch * 128:(ch + 1) * 128, t // 3, t % 3].T
        return wt, b_eff.astype(f32).reshape(128, 1)

    ws_s, bs_s = fold_stem(g['ts_w'], g['ts_b'], g['ts_g'], g['ts_be'], g['ts_m'], g['ts_v'])
    ws_q, bs_q = fold_stem(g['tq_w'], g['tq_b'], g['tq_g'], g['tq_be'], g['tq_m'], g['tq_v'])

    s_cc = g['cc_g'] / np.sqrt(g['cc_v'] + eps)
    wcc_eff = g['cc_w'] * s_cc[:, None, None, None]     # [128, 256, 3, 3]
    bcc_eff = (g['cc_be'] - g['cc_m'] * s_cc).astype(f32)
    # role S convolves E_s (input channels 128:256); role Q convolves E_q (0:128)
    wcc_role = {}
    for role, c0 in (('s', 128), ('q', 0)):
        wt = np.zeros((9, 128, 128), f32)
        for t in range(9):
            wt[t] = wcc_eff[:, c0:c0 + 128, t // 3, t % 3].T
        wcc_role[role] = wt

    wv = np.ascontiguousarray(g['cv_w'][:, :, 0, 0].T)         # [ci, c]
    bvrow = g['cv_b'].reshape(1, 128)
    wk1 = np.ascontiguousarray(g['k1_w'][:, :, 0, 0].T)        # [ci, 64]
    wk2n = np.ascontiguousarray((-g['k2_w'][:, :, 0, 0]).T)
    wq1 = np.ascontiguousarray(g['q1_w'][:, :, 0, 0].T)
    wq2 = np.ascontiguousarray(g['q2_w'][:, :, 0, 0].T)
    w1t = np.ascontiguousarray(g['g1_w'].T) / float(N)         # [128, 8] mean-folded
    b1 = g['g1_b'].reshape(CR, 1)
    w2t = np.ascontiguousarray(g['g2_w'].T)                    # [8, 128]
    b2 = g['g2_b'].reshape(128, 1)

    def pad_img(x, h):                                          # x [256, 64, 64]
        out = np.zeros((256, NROW_IN, PW), f32)
        r0, r1 = 32 * h - 1, 32 * h + 33
        cr0, cr1 = max(r0, 0), min(r1, H)
        out[:, cr0 - r0:cr1 - r0, 1:65] = x[:, cr0:cr1, :]
        return out.reshape(2, 128, NROW_IN * PW)

    P = np.arange(128)
    def rows(sec, slot, chan_off=0):
        # AG2a table: 128-row sections [k:0, q:64]; AG2b: 256-row sections [v:0, st:128]
        if slot in ('k', 'q'):
            base = 128 * sec + (0 if slot == 'k' else 64)
        else:
            base = 256 * sec + (0 if slot == 'v' else 128)
        return (base + chan_off + P).astype(np.int32).reshape(128, 1)

    def rows64(sec_lo, slot, sec_hi):
        lo = rows(sec_lo, slot)[0:64, 0]
        hi = rows(sec_hi, slot)[0:64, 0]
        return np.concatenate([lo, hi]).astype(np.int32).reshape(128, 1)

    in_maps = []
    for c in range(8):
        b, r = c // 4, c % 4
        role_s = r < 2
        h = r % 2
        img_full = g['s'][b] if role_s else g['q'][b]
        d = {
            'img': pad_img(img_full, h),
            'wstem': ws_s if role_s else ws_q,
            'bstem': bs_s if role_s else bs_q,
            'wkp': wk1 if role_s else wk2n,
            'bkp': (g['k1_b'] if role_s else -g['k2_b']).reshape(CH, 1),
            'wqp': wq1 if role_s else wq2,
            'bqp': (g['q1_b'] if role_s else g['q2_b']).reshape(CH, 1),
            'wv': wv, 'bvrow': bvrow,
            'w1t': w1t, 'b1': b1, 'w2t': w2t, 'b2': b2,
            'bcc': bcc_eff.reshape(128, 1),
        }
        d['wcc'] = wcc_role['s' if role_s else 'q']
        # gather indices (sections: 0,1 = s-img halves; 2,3 = q-img halves).
        # u-chunk order is LOCAL-half-first: x1a covers my own pixel half so
        # PV steps 0..15 can run on the locally computed v before the v
        # exchange lands.
        slot1 = 'q' if role_s else 'k'
        d['ix_x1a'] = rows64(h, slot1, 2 + h)
        d['ix_x1b'] = rows64(1 - h, slot1, 2 + (1 - h))
        d['ix_x2'] = rows64(h, 'k' if role_s else 'q', 2 + h)
        osecs = (2, 3) if role_s else (0, 1)
        # z-partial granule of section `osec` that covers MY pixels: granule 0
        # holds osec's own half (its local-first order), granule 1 the other.
        d['ix_d1'] = ((128 * osecs[0] + P) * 2 + h).astype(np.int32).reshape(128, 1)
        d['ix_d2'] = ((128 * osecs[1] + P) * 2 + (1 - h)).astype(np.int32).reshape(128, 1)
        # partner v rows in the 2-rank ag2b table (my rank within pair = h)
        d['ix_vp'] = (128 * (1 - h) + P).astype(np.int32).reshape(128, 1)
        # halo gathers from the 2-rank halo table ([256 rows, 3 granules of
        # 64]): rank within the same-role pair = h; granule 0 = top row of
        # that core's E half, 1 = bottom row, 2 = zeros
        myrk, prk = h, 1 - h
        if h == 0:
            top = (128 * myrk + P) * 3 + 2           # zeros (image row -1)
            bot = (128 * prk + P) * 3 + 0            # partner's first row (row 32)
        else:
            top = (128 * prk + P) * 3 + 1            # partner's last row (row 31)
            bot = (128 * myrk + P) * 3 + 2           # zeros (image row 64)
        d['ix_h1'] = top.astype(np.int32).reshape(128, 1)
        d['ix_h2'] = bot.astype(np.int32).reshape(128, 1)
        # cc-partial partner = other rank of the 2-rank cross-role pair
        pprk = 1 - (0 if r < 2 else 1)
        d['ix_pp'] = (128 * pprk + P).astype(np.int32).reshape(128, 1)
        in_maps.append(d)
    return in_maps


def _assemble(results):
    cpam = np.zeros((B, C, H, W), np.float32)
    e_q = np.zeros((B, C, H, W), np.float32)
    e_s = np.zeros((B, C, H, W), np.float32)
    for c in range(8):
        b, r = c // 4, c % 4
        h = r % 2
        e_half = results[c]['o_e'].reshape(C, 32, W)
        tgt = e_s if r < 2 else e_q
        tgt[b, :, 32 * h:32 * h + 32, :] = e_half
        if r < 2:
            cpam[b, :, 32 * h:32 * h + 32, :] = results[c]['o_cc'].reshape(128, 32, W)
    return cpam, e_q, e_s


def kernel(**inputs):
    nc = build_program()
    in_maps = _prep_inputs(inputs)
    res = bass_utils.run_bass_kernel_spmd(nc, in_maps, core_ids=list(range(8)))
    return _assemble(res.results)


def kernel_traced(**inputs):
    """Like kernel() but reporting a time estimate.

    Tries NTFF tracing (real HW exec time); if the profiling hook is not
    available in this environment, falls back to the Tile cost-model
    timeline simulation (single-core makespan; its collective cost model
    assumes cross-chip scale, so it substantially over-estimates the
    intra-chip AllGathers this kernel uses).
    """
    nc = build_program()
    in_maps = _prep_inputs(inputs)
    exec_ns = None
    try:
        res = bass_utils.run_bass_kernel_spmd(nc, in_maps, core_ids=list(range(8)),
                                              trace=True)
        exec_ns = res.exec_time_ns
    except Exception:
        res = bass_utils.run_bass_kernel_spmd(nc, in_maps, core_ids=list(range(8)))
    if exec_ns is None:
        try:
            from concourse.timeline_sim import TimelineSim
            exec_ns = int(TimelineSim(nc, no_exec=True, trace=False).simulate())
        except Exception:
            exec_ns = -1
    return _assemble(res.results), exec_ns



# revision 50
# speedup vs baseline: 1.0594x; 1.0594x over previous
"""Trainium2 Bass kernel for nn_FEM_best (dual-attention fusion module).

Decomposition over 8 NeuronCores: core c = b*4 + r, b in {0,1}, r in:
  r=0: role S (computes E_s), pixel half 0   (phase-1: s-image stem, half 0)
  r=1: role S, half 1                        (phase-1: s-image stem, half 1)
  r=2: role Q (computes E_q), half 0         (phase-1: q-image stem, half 0)
  r=3: role Q, half 1                        (phase-1: q-image stem, half 1)
plus a channel-quarter of the final concat-conv on every core.

Single SPMD program; all per-core asymmetry is carried by input data
(weights, pre-padded image halves, and row-gather index tensors).
Cross-core exchange via AllGather collectives through DRAM bounce
tables + indirect row-gather DMAs.
"""
import sys, os
sys.path.insert(0, '/opt/trn_rl_repo')
import numpy as np

import concourse.bass as bass
import concourse.mybir as mybir
import concourse.bacc as bacc
import concourse.tile as tile
from concourse import bass_utils

F32 = mybir.dt.float32
F32R = mybir.dt.float32r
I32 = mybir.dt.int32
AF = mybir.ActivationFunctionType
ALU = mybir.AluOpType

B, CIN, H, W = 2, 256, 64, 64
N = H * W                 # 4096
C = 128                   # inter channels
CH = 64                   # C//2 (k/q projection channels)
CR = 8                    # gate bottleneck
HALF = N // 2             # 2048
PW = W + 2                # padded row width 66
NROW_IN = 34              # input rows per half (32 + 2 halo)
TC = 512                  # tile free size
NJ = HALF // TC           # 4 t-chunks per half
NU = N // 128             # 32 u-chunks
EXP_BIAS = -40.0
BF16 = mybir.dt.bfloat16

_cache = {}


def build_program():
    if 'nc' in _cache:
        return _cache['nc']
    nc = bacc.Bacc("TRN2", target_bir_lowering=False, debug=False, num_devices=8)

    def din(name, shape, dt=F32):
        return nc.dram_tensor(name, list(shape), dt, kind="ExternalInput").ap()

    def dout(name, shape, dt=F32):
        return nc.dram_tensor(name, list(shape), dt, kind="ExternalOutput").ap()

    i_img = din("img", [2, 128, NROW_IN * PW])          # padded input image half, 2 cin chunks
    i_wstem = din("wstem", [2, 9, 128, 128])            # (chunk, tap, ci, co) BN-folded
    i_bstem = din("bstem", [128, 1])
    i_wkp = din("wkp", [128, CH])                       # k-part projection (sign-folded)
    i_bkp = din("bkp", [CH, 1])
    i_wqp = din("wqp", [128, CH])
    i_bqp = din("bqp", [CH, 1])
    i_wv = din("wv", [128, 128])                        # cv_w^T [ci, c]
    i_bvrow = din("bvrow", [1, 128])                    # cv_b as a row
    i_wcc = din("wcc", [9, 128, 128])                   # cc conv, my role's cin half
    i_bcc = din("bcc", [128, 1])
    i_w1t = din("w1t", [128, CR])                       # gate MLP (mean-folded)
    i_b1 = din("b1", [CR, 1])
    i_w2t = din("w2t", [CR, 128])
    i_b2 = din("b2", [128, 1])
    ix_x1a = din("ix_x1a", [128, 1], I32)               # row-gather indices into AG2 table
    ix_x1b = din("ix_x1b", [128, 1], I32)
    ix_x2 = din("ix_x2", [128, 1], I32)
    ix_vp = din("ix_vp", [128, 1], I32)                 # partner v rows in ag2b table
    ix_d1 = din("ix_d1", [128, 1], I32)                 # fine-row indices into AG3 table
    ix_d2 = din("ix_d2", [128, 1], I32)
    ix_h1 = din("ix_h1", [128, 1], I32)                 # top-halo row gather
    ix_h2 = din("ix_h2", [128, 1], I32)                 # bottom-halo row gather
    ix_pp = din("ix_pp", [64, 1], I32)                  # partner cc-partial gather

    o_e = dout("o_e", [128, HALF])                      # E_{s|q} half (channel-major)
    o_cc = dout("o_cc", [64, HALF])                     # cpam pixel half, my co-half

    with tile.TileContext(nc) as tc:
        with tc.tile_pool(name="per", bufs=1) as per, \
             tc.tile_pool(name="dram", bufs=1, space="DRAM") as dram:
            # ---- persistent tiles ----
            x1a = per.tile([128, HALF], F32R)
            x1b = per.tile([128, HALF], F32R)
            x2 = per.tile([128, HALF], F32R)
            vpm = per.tile([128, HALF], BF16)            # my v, pixel-major (u 0..15)
            vsec = per.tile([128, HALF], BF16)           # partner-half v (u 16..31)
            stem = per.tile([128, HALF], F32R)           # my stem output (= resid)
            gvec = per.tile([128, 1], F32)
            zslot = per.tile([128, NU], F32)
            etile = per.tile([128, HALF], F32)
            rbc = per.tile([128, HALF], F32)
            bneg = per.tile([128, 1], F32)
            nc.gpsimd.memset(bneg[:], EXP_BIAS)
            ones_u = per.tile([128, 1], BF16)
            nc.gpsimd.memset(ones_u[:], 1.0)

            ag2a_in = dram.tile([128, 2048], F32)
            ag2a_out = dram.tile([512, 2048], F32)
            ag2b_in = dram.tile([128, 2048], BF16)       # my v (pixel-major), bf16
            ag2b_out = dram.tile([256, 2048], BF16)      # 2-rank: [half0 v; half1 v]
            ag3_in = dram.tile([128, 32], F32)
            ag3_out = dram.tile([512, 32], F32)
            agh_in = dram.tile([128, 192], F32)          # halo: top row | bottom row | zeros
            agh_out = dram.tile([256, 192], F32)         # 2-rank same-role pair
            agp_in = dram.tile([64, 2048], BF16)         # partner's co-half partial
            agp_out = dram.tile([128, 2048], BF16)       # 2-rank cross-role pair
            dsc = dram.tile([16, 128], F32)             # transpose scratch for D row

            # ================= Phase A: stem + projections =================
            with nc.named_scope("pA_stem"), \
                 tc.tile_pool(name="pha", bufs=1) as pha, \
                 tc.tile_pool(name="psA", bufs=2, space="PSUM") as psA:
                # weights first so the first stem matmul isn't DMA-queued
                wstem = pha.tile([128, 2, 9, 128], F32R)
                nc.sync.dma_start(wstem[:], i_wstem[:].rearrange("a t p c -> p a t c").bitcast(F32R))
                bstem = pha.tile([128, 1], F32)
                nc.sync.dma_start(bstem[:], i_bstem[:])
                wkp = pha.tile([128, CH], F32R)
                nc.sync.dma_start(wkp[:], i_wkp[:].bitcast(F32R))
                bkp = pha.tile([CH, 1], F32)
                nc.sync.dma_start(bkp[:], i_bkp[:])
                wqp = pha.tile([128, CH], F32R)
                nc.sync.dma_start(wqp[:], i_wqp[:].bitcast(F32R))
                bqp = pha.tile([CH, 1], F32)
                nc.sync.dma_start(bqp[:], i_bqp[:])
                img = pha.tile([128, 2, NROW_IN * PW + 2], F32R)
                isrc = i_img[:].rearrange("a p x -> p a x").bitcast(F32R)
                for r0, r1 in ((0, 10), (10, 18), (18, 26), (26, 34)):
                    nc.sync.dma_start(img[:, :, r0 * PW:r1 * PW],
                                      isrc[:, :, r0 * PW:r1 * PW])

                kpart = pha.tile([CH, HALF], F32R)
                qpart = pha.tile([CH, HALF], F32R)
                for pc in range(NJ):                    # 4 pixel chunks of 512 (8 img rows)
                    sl = slice(pc * TC, (pc + 1) * TC)
                    ps = psA.tile([128, TC], F32)
                    first = True
                    for ch in range(2):
                        for t in range(9):
                            dy, dx = t // 3 - 1, t % 3 - 1
                            off = (8 * pc + dy + 1) * PW + (dx + 1)
                            rhs = img[:, ch, off:off + 8 * PW].rearrange(
                                "p (r w) -> p r w", r=8)[:, :, 0:64]
                            nc.tensor.matmul(ps[:], wstem[:, ch, t, :], rhs,
                                             start=first, stop=(ch == 1 and t == 8))
                            first = False
                    nc.scalar.activation(stem[:, sl], ps[:], AF.Relu, bias=bstem[:])
                    # fused per-chunk projections + collective payload
                    psk = psA.tile([CH, TC], F32)
                    nc.tensor.matmul(psk[:], wkp[:], stem[:, sl], start=True, stop=True)
                    nc.scalar.activation(kpart[:, sl], psk[:], AF.Identity, bias=bkp[:])
                    psq = psA.tile([CH, TC], F32)
                    nc.tensor.matmul(psq[:], wqp[:], stem[:, sl], start=True, stop=True)
                    nc.scalar.activation(qpart[:, sl], psq[:], AF.Identity, bias=bqp[:])
                    nc.sync.dma_start(ag2a_in[0:64, sl], kpart[:, sl].bitcast(F32))
                    nc.sync.dma_start(ag2a_in[64:128, sl], qpart[:, sl].bitcast(F32))

                with nc.named_scope("ag2a"):
                    nc.gpsimd.collective_compute(
                        "AllGather", ALU.bypass,
                        replica_groups=[[0, 1, 2, 3], [4, 5, 6, 7]],
                        ins=[ag2a_in.opt()], outs=[ag2a_out.opt()],
                    )

                # v (pixel-major, bf16) computed after the kq collective is
                # triggered; its 2-rank exchange rides behind ag2a on the
                # collective engine while scores run.
                wv = pha.tile([128, 128], F32)
                nc.sync.dma_start(wv[:], i_wv[:])
                bvrow = pha.tile([1, 128], F32)
                nc.sync.dma_start(bvrow[:], i_bvrow[:])
                bvbc = pha.tile([128, 128], F32)
                nc.gpsimd.partition_broadcast(bvbc[:], bvrow[:])
                for uc in range(16):
                    psv = psA.tile([128, 128], F32)
                    nc.tensor.matmul(psv[:], stem[:, uc * 128:(uc + 1) * 128].bitcast(F32),
                                     wv[:], start=True, stop=True)
                    nc.vector.tensor_tensor(vpm[:, uc * 128:(uc + 1) * 128],
                                            psv[:], bvbc[:], ALU.add)
                nc.sync.dma_start(ag2b_in[:], vpm[:])

                with nc.named_scope("ag2b"):
                    nc.gpsimd.collective_compute(
                        "AllGather", ALU.bypass,
                        replica_groups=[[0, 1], [2, 3], [4, 5], [6, 7]],
                        ins=[ag2b_in.opt()], outs=[ag2b_out.opt()],
                    )

            # ================= Phase B: gathers =================
            with nc.named_scope("pB_gather"), tc.tile_pool(name="phb", bufs=1) as phb:
                def gather(table, dst, idx_dram, cast=True):
                    idxt = phb.tile([128, 1], I32, name=f"idx_{dst.tensor.name}")
                    nc.sync.dma_start(idxt[:], idx_dram[:])
                    srcv = table[:].bitcast(F32R) if cast else table[:]
                    nc.gpsimd.indirect_dma_start(
                        out=dst[:], out_offset=None, in_=srcv,
                        in_offset=bass.IndirectOffsetOnAxis(ap=idxt[:, :1], axis=0))

                gather(ag2a_out, x1a, ix_x1a)
                gather(ag2a_out, x2, ix_x2)
                gather(ag2a_out, x1b, ix_x1b)
                gather(ag2b_out, vsec, ix_vp, cast=False)

            # prefetch concat-conv weights during attention
            wcc = per.tile([128, 9, 128], F32R)
            nc.sync.dma_start(wcc[:], i_wcc[:].rearrange("t p c -> p t c").bitcast(F32R))
            bcc = per.tile([128, 1], F32)
            nc.sync.dma_start(bcc[:], i_bcc[:])

            # ================= Phase C: attention =================
            with nc.named_scope("pC_attn"), tc.tile_pool(name="phc", bufs=3) as phc, \
                 tc.tile_pool(name="psS", bufs=3, space="PSUM") as psS, \
                 tc.tile_pool(name="psO", bufs=1, space="PSUM") as psO:
                pv_ps = []
                for j in range(NJ):
                    pv_ps.append(psO.tile([128, TC], F32, name=f"pvps{j}"))
                ACT_ABS_U = set()      # ACT is the pace-setter now; abs all on DVE
                LAG = 6
                pts = {}
                deferred = []

                def emit_pv(uu):
                    vt = vpm if uu < 16 else vsec
                    uslv = slice((uu % 16) * 128, (uu % 16) * 128 + 128)
                    ptv = pts.pop(uu)
                    for j in range(NJ):
                        tsl = slice(j * TC, (j + 1) * TC)
                        nc.tensor.matmul(pv_ps[j][:], vt[:, uslv], ptv[:, tsl],
                                         start=(uu == 0), stop=(uu == NU - 1))

                for step in range(NU + LAG):
                    if step < NU:
                        u = step
                        x1t = x1a if u < 16 else x1b
                        usl = slice((u % 16) * 128, (u % 16) * 128 + 128)
                        pabs = phc.tile([128, HALF], F32, name="pabs", bufs=4)
                        for j2 in range(2):
                            t2 = slice(j2 * 2 * TC, (j2 * 2 + 2) * TC)
                            sps = psS.tile([128, 2 * TC], F32, name="sps", bufs=2)
                            for jj in range(2):
                                nc.tensor.matmul(
                                    sps[:, jj * TC:(jj + 1) * TC], x1t[:, usl],
                                    x2[:, (j2 * 2 + jj) * TC:(j2 * 2 + jj + 1) * TC],
                                    start=True, stop=True)
                            if u in ACT_ABS_U:
                                nc.scalar.activation(pabs[:, t2], sps[:], AF.Abs)
                            else:
                                nc.vector.tensor_scalar(pabs[:, t2].bitcast(I32),
                                                        sps[:].bitcast(I32),
                                                        0x7FFFFFFF, None, ALU.bitwise_and)
                        pt = phc.tile([128, HALF], BF16, name="p_t", bufs=LAG + 2)
                        nc.scalar.activation(pt[:], pabs[:], AF.Exp, bias=bneg[:],
                                             accum_out=zslot[:, u:u + 1])
                        pts[u] = pt
                    if step == NU:
                        # gate: sigmoid(w2 @ relu(w1 @ mean(v) + b1) + b2), mean folded in w1
                        w1t = phc.tile([128, CR], F32, name="w1t")
                        nc.sync.dma_start(w1t[:], i_w1t[:])
                        b1 = phc.tile([CR, 1], F32, name="b1")
                        nc.sync.dma_start(b1[:], i_b1[:])
                        w2t = phc.tile([CR, 128], F32, name="w2t")
                        nc.sync.dma_start(w2t[:], i_w2t[:])
                        b2 = phc.tile([128, 1], F32, name="b2")
                        nc.sync.dma_start(b2[:], i_b2[:])
                        psum_v = psS.tile([128, 1], F32, name="psum_v", tag="sps", bufs=2)
                        for half, vt in ((0, vpm), (1, vsec)):
                            for uc in range(16):
                                nc.tensor.matmul(psum_v[:],
                                                 vt[:, uc * 128:(uc + 1) * 128],
                                                 ones_u[:], start=(half == 0 and uc == 0),
                                                 stop=(half == 1 and uc == 15))
                        vsum = phc.tile([128, 1], F32, name="vsum")
                        nc.scalar.copy(vsum[:], psum_v[:])
                        psh = psS.tile([CR, 1], F32, name="psh", tag="sps", bufs=2)
                        nc.tensor.matmul(psh[:], w1t[:], vsum[:], start=True, stop=True)
                        hgate = phc.tile([CR, 1], F32, name="hgate")
                        nc.scalar.activation(hgate[:], psh[:], AF.Relu, bias=b1[:])
                        psg = psS.tile([128, 1], F32, name="psg", tag="sps", bufs=2)
                        nc.tensor.matmul(psg[:], w2t[:], hgate[:], start=True, stop=True)
                        nc.scalar.activation(gvec[:], psg[:], AF.Sigmoid, bias=b2[:])
                    if step >= LAG:
                        uu = step - LAG
                        # partner-half PVs (u>=16) are deferred so their
                        # weight-loads never block the in-order PE queue
                        # before the v exchange lands
                        if uu < 16:
                            emit_pv(uu)
                        else:
                            deferred.append(uu)
                        if step >= 22:
                            for _ in range(2):
                                if deferred:
                                    emit_pv(deferred.pop(0))
                while deferred:
                    emit_pv(deferred.pop(0))

                nc.sync.dma_start(ag3_in[:], zslot[:])

                with nc.named_scope("ag3"):
                    nc.gpsimd.collective_compute(
                        "AllGather", ALU.bypass,
                        replica_groups=[[0, 1, 2, 3], [4, 5, 6, 7]],
                        ins=[ag3_in.opt()], outs=[ag3_out.opt()],
                    )

                # D = sum of the two other-role partials, my half (fine rows of 16)
                with nc.named_scope("pD_efin"):
                    ag3f = ag3_out[:].rearrange("r (g w) -> (r g) w", w=16)
                    w1g = phc.tile([128, 16], F32, name="w1g")
                    w2g = phc.tile([128, 16], F32, name="w2g")
                    for dst, ixd in ((w1g, ix_d1), (w2g, ix_d2)):
                        idxt = phc.tile([128, 1], I32, name=f"ixd_{dst.tensor.name}")
                        nc.sync.dma_start(idxt[:], ixd[:])
                        nc.gpsimd.indirect_dma_start(
                            out=dst[:], out_offset=None, in_=ag3f,
                            in_offset=bass.IndirectOffsetOnAxis(ap=idxt[:, :1], axis=0))
                    dmine = phc.tile([128, 16], F32, name="dmine")
                    nc.vector.tensor_tensor(dmine[:], w1g[:], w2g[:], ALU.add)
                    rrec = phc.tile([128, 16], F32, name="rrec")
                    nc.vector.reciprocal(rrec[:], dmine[:])
                    nc.sync.dma_start(dsc[:].rearrange("c p -> p c"), rrec[:])
                    drow = phc.tile([1, HALF], F32, name="drow")
                    nc.sync.dma_start(drow[:], dsc[:].rearrange("c p -> (c p)").unsqueeze(0))
                    nc.gpsimd.partition_broadcast(rbc[:], drow[:])

                    # E = (PV * g) * R + resid; edge chunks first so the halo
                    # exchange can start while the interior finishes
                    halo = phc.tile([128, 192], F32, name="halo")
                    nc.vector.memset(halo[:, 128:192], 0.0)
                    for j in (0, 3, 1, 2):
                        tsl = slice(j * TC, (j + 1) * TC)
                        nc.vector.scalar_tensor_tensor(etile[:, tsl], pv_ps[j][:],
                                                       gvec[:], rbc[:, tsl],
                                                       ALU.mult, ALU.mult)
                        nc.vector.tensor_tensor(etile[:, tsl], etile[:, tsl],
                                                stem[:, tsl].bitcast(F32), ALU.add)
                        if j == 0:
                            nc.vector.tensor_copy(halo[:, 0:64], etile[:, 0:64])
                        elif j == 3:
                            nc.vector.tensor_copy(halo[:, 64:128],
                                                  etile[:, HALF - 64:HALF])
                            nc.sync.dma_start(agh_in[:], halo[:])
                    for j in range(NJ):
                        tsl = slice(j * TC, (j + 1) * TC)
                        nc.sync.dma_start(o_e[:, tsl], etile[:, tsl])

            with nc.named_scope("agh"):
                nc.gpsimd.collective_compute(
                    "AllGather", ALU.bypass,
                    replica_groups=[[0, 1], [2, 3], [4, 5], [6, 7]],
                    ins=[agh_in.opt()], outs=[agh_out.opt()],
                )

            # ============ Phase E: concat conv via per-role partials ============
            with nc.named_scope("pE_cc"), tc.tile_pool(name="phe", bufs=1) as phe, \
                 tc.tile_pool(name="psE", bufs=2, space="PSUM") as psE:
                EPW = NROW_IN * PW + 2                   # 34 padded rows of 66 (+slack)
                epadS = phe.tile([128, EPW], F32R)       # my E half, padded
                zsrc = phe.tile([128, PW], F32R)
                nc.vector.memset(zsrc[:].bitcast(F32), 0.0)
                # interior rows 1..32 <- my E (restride 64 -> 66)
                nc.sync.dma_start(
                    epadS[:, PW + 1:33 * PW + 1].rearrange("p (r w) -> p r w", w=PW)[:, :, 0:64],
                    etile[:].rearrange("p (r w) -> p r w", w=64).bitcast(F32R))
                # halo rows 0 and 33 via indexed gather from the halo table
                aghf = agh_out[:].rearrange("r (g w) -> (r g) w", w=64).bitcast(F32R)
                for ixd, row in ((ix_h1, 0), (ix_h2, 33)):
                    idxt = phe.tile([128, 1], I32, name=f"ixh_{row}")
                    nc.sync.dma_start(idxt[:], ixd[:])
                    nc.gpsimd.indirect_dma_start(
                        out=epadS[:, row * PW + 1:row * PW + 65], out_offset=None,
                        in_=aghf,
                        in_offset=bass.IndirectOffsetOnAxis(ap=idxt[:, :1], axis=0))
                # zero borders: left/right columns of all 34 rows
                lcol = epadS[:, 0:34 * PW].rearrange("p (r w) -> p r w", w=PW)[:, :, 0:1]
                nc.sync.dma_start(lcol, zsrc[:, 0:34].unsqueeze(2))
                rcol = epadS[:, 65:34 * PW - 1].rearrange("p (r w) -> p r w", w=PW)[:, :, 0:1]
                nc.sync.dma_start(rcol, zsrc[:, 0:33].unsqueeze(2))
                nc.sync.dma_start(epadS[:, 34 * PW - 1:34 * PW + 1], zsrc[:, 0:2])

                pcc = phe.tile([128, HALF], BF16)
                for pc in (1, 2, 0, 3):
                    ps = psE.tile([128, TC], F32)
                    for t in range(9):
                        dy, dx = t // 3 - 1, t % 3 - 1
                        off = (8 * pc + dy + 1) * PW + (dx + 1)
                        rhs = epadS[:, off:off + 8 * PW].rearrange(
                            "p (r w) -> p r w", r=8)[:, :, 0:64]
                        nc.tensor.matmul(ps[:], wcc[:, t, :], rhs,
                                         start=(t == 0), stop=(t == 8))
                    nc.scalar.copy(pcc[:, pc * TC:(pc + 1) * TC], ps[:])
                    nc.sync.dma_start(agp_in[:, pc * TC:(pc + 1) * TC],
                                      pcc[64:128, pc * TC:(pc + 1) * TC])

                with nc.named_scope("agp"):
                    nc.gpsimd.collective_compute(
                        "AllGather", ALU.bypass,
                        replica_groups=[[0, 2], [1, 3], [4, 6], [5, 7]],
                        ins=[agp_in.opt()], outs=[agp_out.opt()],
                    )
                ppart = phe.tile([64, HALF], BF16)
                idxp = phe.tile([64, 1], I32)
                nc.sync.dma_start(idxp[:], ix_pp[:])
                nc.gpsimd.indirect_dma_start(
                    out=ppart[:], out_offset=None, in_=agp_out[:],
                    in_offset=bass.IndirectOffsetOnAxis(ap=idxp[:, :1], axis=0))
                csum = phe.tile([64, HALF], BF16)
                nc.vector.tensor_tensor(csum[:], pcc[0:64, :], ppart[:], ALU.add)
                ccout = phe.tile([64, HALF], F32)
                nc.scalar.activation(ccout[:], csum[:], AF.Relu, bias=bcc[0:64, :])
                nc.sync.dma_start(o_cc[:], ccout[:])

    nc.compile()
    _cache['nc'] = nc
    return nc


# ====================== host-side preparation ======================

def _prep_inputs(inp):
    """Build the 8 per-core input dicts from the full problem inputs."""
    f32 = np.float32
    g = {k: np.asarray(v, f32) for k, v in inp.items()}
    eps = 1e-5

    def fold_stem(w, b, gam, be, m, v):
        s = gam / np.sqrt(v + eps)
        w_eff = w * s[:, None, None, None]                     # [co, cin, 3, 3]
        b_eff = (b - m) * s + be
        # [2, 9, 128, 128] : (cin chunk, tap, ci, co)
        wt = np.zeros((2, 9, 128, 128), f32)
        for ch in range(2):
            for t in range(9):
                wt[ch, t] = w_eff[:, ch * 128:(ch + 1) * 128, t // 3, t % 3].T
        return wt, b_eff.astype(f32).reshape(128, 1)

    ws_s, bs_s = fold_stem(g['ts_w'], g['ts_b'], g['ts_g'], g['ts_be'], g['ts_m'], g['ts_v'])
    ws_q, bs_q = fold_stem(g['tq_w'], g['tq_b'], g['tq_g'], g['tq_be'], g['tq_m'], g['tq_v'])

    s_cc = g['cc_g'] / np.sqrt(g['cc_v'] + eps)
    wcc_eff = g['cc_w'] * s_cc[:, None, None, None]     # [128, 256, 3, 3]
    bcc_eff = (g['cc_be'] - g['cc_m'] * s_cc).astype(f32)
    # role S convolves E_s (input channels 128:256); role Q convolves E_q
    # (0:128). Output channels are permuted so rows 0:64 are always the
    # co-half this core finalizes (S: 0:64, Q: 64:128) and rows 64:128 the
    # half it ships to its cross-role partner.
    co_perm = {'s': np.arange(128),
               'q': np.concatenate([np.arange(64, 128), np.arange(0, 64)])}
    wcc_role = {}
    bcc_role = {}
    for role, c0 in (('s', 128), ('q', 0)):
        perm = co_perm[role]
        wt = np.zeros((9, 128, 128), f32)
        for t in range(9):
            wt[t] = wcc_eff[perm][:, c0:c0 + 128, t // 3, t % 3].T
        wcc_role[role] = wt
        bcc_role[role] = bcc_eff[perm].reshape(128, 1)

    wv = np.ascontiguousarray(g['cv_w'][:, :, 0, 0].T)         # [ci, c]
    bvrow = g['cv_b'].reshape(1, 128)
    wk1 = np.ascontiguousarray(g['k1_w'][:, :, 0, 0].T)        # [ci, 64]
    wk2n = np.ascontiguousarray((-g['k2_w'][:, :, 0, 0]).T)
    wq1 = np.ascontiguousarray(g['q1_w'][:, :, 0, 0].T)
    wq2 = np.ascontiguousarray(g['q2_w'][:, :, 0, 0].T)
    w1t = np.ascontiguousarray(g['g1_w'].T) / float(N)         # [128, 8] mean-folded
    b1 = g['g1_b'].reshape(CR, 1)
    w2t = np.ascontiguousarray(g['g2_w'].T)                    # [8, 128]
    b2 = g['g2_b'].reshape(128, 1)

    def pad_img(x, h):                                          # x [256, 64, 64]
        out = np.zeros((256, NROW_IN, PW), f32)
        r0, r1 = 32 * h - 1, 32 * h + 33
        cr0, cr1 = max(r0, 0), min(r1, H)
        out[:, cr0 - r0:cr1 - r0, 1:65] = x[:, cr0:cr1, :]
        return out.reshape(2, 128, NROW_IN * PW)

    P = np.arange(128)
    def rows(sec, slot, chan_off=0):
        # AG2a table: 128-row sections [k:0, q:64]; AG2b: 256-row sections [v:0, st:128]
        if slot in ('k', 'q'):
            base = 128 * sec + (0 if slot == 'k' else 64)
        else:
            base = 256 * sec + (0 if slot == 'v' else 128)
        return (base + chan_off + P).astype(np.int32).reshape(128, 1)

    def rows64(sec_lo, slot, sec_hi):
        lo = rows(sec_lo, slot)[0:64, 0]
        hi = rows(sec_hi, slot)[0:64, 0]
        return np.concatenate([lo, hi]).astype(np.int32).reshape(128, 1)

    in_maps = []
    for c in range(8):
        b, r = c // 4, c % 4
        role_s = r < 2
        h = r % 2
        img_full = g['s'][b] if role_s else g['q'][b]
        d = {
            'img': pad_img(img_full, h),
            'wstem': ws_s if role_s else ws_q,
            'bstem': bs_s if role_s else bs_q,
            'wkp': wk1 if role_s else wk2n,
            'bkp': (g['k1_b'] if role_s else -g['k2_b']).reshape(CH, 1),
            'wqp': wq1 if role_s else wq2,
            'bqp': (g['q1_b'] if role_s else g['q2_b']).reshape(CH, 1),
            'wv': wv, 'bvrow': bvrow,
            'w1t': w1t, 'b1': b1, 'w2t': w2t, 'b2': b2,
        }
        d['wcc'] = wcc_role['s' if role_s else 'q']
        d['bcc'] = bcc_role['s' if role_s else 'q']
        # gather indices (sections: 0,1 = s-img halves; 2,3 = q-img halves).
        # u-chunk order is LOCAL-half-first: x1a covers my own pixel half so
        # PV steps 0..15 can run on the locally computed v before the v
        # exchange lands.
        slot1 = 'q' if role_s else 'k'
        d['ix_x1a'] = rows64(h, slot1, 2 + h)
        d['ix_x1b'] = rows64(1 - h, slot1, 2 + (1 - h))
        d['ix_x2'] = rows64(h, 'k' if role_s else 'q', 2 + h)
        osecs = (2, 3) if role_s else (0, 1)
        # z-partial granule of section `osec` that covers MY pixels: granule 0
        # holds osec's own half (its local-first order), granule 1 the other.
        d['ix_d1'] = ((128 * osecs[0] + P) * 2 + h).astype(np.int32).reshape(128, 1)
        d['ix_d2'] = ((128 * osecs[1] + P) * 2 + (1 - h)).astype(np.int32).reshape(128, 1)
        # partner v rows in the 2-rank ag2b table (my rank within pair = h)
        d['ix_vp'] = (128 * (1 - h) + P).astype(np.int32).reshape(128, 1)
        # halo gathers from the 2-rank halo table ([256 rows, 3 granules of
        # 64]): rank within the same-role pair = h; granule 0 = top row of
        # that core's E half, 1 = bottom row, 2 = zeros
        myrk, prk = h, 1 - h
        if h == 0:
            top = (128 * myrk + P) * 3 + 2           # zeros (image row -1)
            bot = (128 * prk + P) * 3 + 0            # partner's first row (row 32)
        else:
            top = (128 * prk + P) * 3 + 1            # partner's last row (row 31)
            bot = (128 * myrk + P) * 3 + 2           # zeros (image row 64)
        d['ix_h1'] = top.astype(np.int32).reshape(128, 1)
        d['ix_h2'] = bot.astype(np.int32).reshape(128, 1)
        # cc-partial partner = other rank of the 2-rank cross-role pair
        pprk = 1 - (0 if r < 2 else 1)
        d['ix_pp'] = (64 * pprk + np.arange(64)).astype(np.int32).reshape(64, 1)
        in_maps.append(d)
    return in_maps


def _assemble(results):
    cpam = np.zeros((B, C, H, W), np.float32)
    e_q = np.zeros((B, C, H, W), np.float32)
    e_s = np.zeros((B, C, H, W), np.float32)
    for c in range(8):
        b, r = c // 4, c % 4
        h = r % 2
        e_half = results[c]['o_e'].reshape(C, 32, W)
        tgt = e_s if r < 2 else e_q
        tgt[b, :, 32 * h:32 * h + 32, :] = e_half
        co0 = 0 if r < 2 else 64
        cpam[b, co0:co0 + 64, 32 * h:32 * h + 32, :] = \
            results[c]['o_cc'].reshape(64, 32, W)
    return cpam, e_q, e_s


def kernel(**inputs):
    nc = build_program()
    in_maps = _prep_inputs(inputs)
    res = bass_utils.run_bass_kernel_spmd(nc, in_maps, core_ids=list(range(8)))
    return _assemble(res.results)


def kernel_traced(**inputs):
    """Like kernel() but reporting a time estimate.

    Tries NTFF tracing (real HW exec time); if the profiling hook is not
    available in this environment, falls back to the Tile cost-model
    timeline simulation (single-core makespan; its collective cost model
    assumes cross-chip scale, so it substantially over-estimates the
    intra-chip AllGathers this kernel uses).
    """
    nc = build_program()
    in_maps = _prep_inputs(inputs)
    exec_ns = None
    try:
        res = bass_utils.run_bass_kernel_spmd(nc, in_maps, core_ids=list(range(8)),
                                              trace=True)
        exec_ns = res.exec_time_ns
    except Exception:
        res = bass_utils.run_bass_kernel_spmd(nc, in_maps, core_ids=list(range(8)))
    if exec_ns is None:
        try:
            from concourse.timeline_sim import TimelineSim
            exec_ns = int(TimelineSim(nc, no_exec=True, trace=False).simulate())
        except Exception:
            exec_ns = -1
    return _assemble(res.results), exec_ns



# revision 52
# speedup vs baseline: 1.1582x; 1.0932x over previous
"""Trainium2 Bass kernel for nn_FEM_best (dual-attention fusion module).

Decomposition over 8 NeuronCores: core c = b*4 + r, b in {0,1}, r in:
  r=0: role S (computes E_s), pixel half 0   (phase-1: s-image stem, half 0)
  r=1: role S, half 1                        (phase-1: s-image stem, half 1)
  r=2: role Q (computes E_q), half 0         (phase-1: q-image stem, half 0)
  r=3: role Q, half 1                        (phase-1: q-image stem, half 1)
plus a channel-quarter of the final concat-conv on every core.

Single SPMD program; all per-core asymmetry is carried by input data
(weights, pre-padded image halves, and row-gather index tensors).
Cross-core exchange via AllGather collectives through DRAM bounce
tables + indirect row-gather DMAs.
"""
import sys, os
sys.path.insert(0, '/opt/trn_rl_repo')
import numpy as np

import concourse.bass as bass
import concourse.mybir as mybir
import concourse.bacc as bacc
import concourse.tile as tile
from concourse import bass_utils

F32 = mybir.dt.float32
F32R = mybir.dt.float32r
I32 = mybir.dt.int32
AF = mybir.ActivationFunctionType
ALU = mybir.AluOpType

B, CIN, H, W = 2, 256, 64, 64
N = H * W                 # 4096
C = 128                   # inter channels
CH = 64                   # C//2 (k/q projection channels)
CR = 8                    # gate bottleneck
HALF = N // 2             # 2048
PW = W + 2                # padded row width 66
NROW_IN = 34              # input rows per half (32 + 2 halo)
TC = 512                  # tile free size
NJ = HALF // TC           # 4 t-chunks per half
NU = N // 128             # 32 u-chunks
EXP_BIAS = -40.0
BF16 = mybir.dt.bfloat16

_cache = {}


def build_program():
    if 'nc' in _cache:
        return _cache['nc']
    nc = bacc.Bacc("TRN2", target_bir_lowering=False, debug=False, num_devices=8)

    def din(name, shape, dt=F32):
        return nc.dram_tensor(name, list(shape), dt, kind="ExternalInput").ap()

    def dout(name, shape, dt=F32):
        return nc.dram_tensor(name, list(shape), dt, kind="ExternalOutput").ap()

    i_img = din("img", [2, 128, NROW_IN * PW])          # padded input image half, 2 cin chunks
    i_wstem = din("wstem", [2, 9, 128, 128])            # (chunk, tap, ci, co) BN-folded
    i_bstem = din("bstem", [128, 1])
    i_wkp = din("wkp", [128, CH])                       # k-part projection (sign-folded)
    i_bkp = din("bkp", [CH, 1])
    i_wqp = din("wqp", [128, CH])
    i_bqp = din("bqp", [CH, 1])
    i_wv = din("wv", [128, 128])                        # cv_w^T [ci, c]
    i_bvrow = din("bvrow", [1, 128])                    # cv_b as a row
    i_wcc = din("wcc", [9, 128, 128])                   # cc conv, my role's cin half
    i_bcc = din("bcc", [128, 1])
    i_w1t = din("w1t", [128, CR])                       # gate MLP (mean-folded)
    i_b1 = din("b1", [CR, 1])
    i_w2t = din("w2t", [CR, 128])
    i_b2 = din("b2", [128, 1])
    ix_x1a = din("ix_x1a", [128, 1], I32)               # row-gather indices into AG2 table
    ix_x1b = din("ix_x1b", [128, 1], I32)
    ix_x2 = din("ix_x2", [128, 1], I32)
    ix_vp = din("ix_vp", [128, 1], I32)                 # partner v rows in ag2b table
    ix_d1 = din("ix_d1", [128, 1], I32)                 # fine-row indices into AG3 table
    ix_d2 = din("ix_d2", [128, 1], I32)
    ix_h1 = din("ix_h1", [128, 1], I32)                 # top-halo row gather
    ix_h2 = din("ix_h2", [128, 1], I32)                 # bottom-halo row gather
    ix_pp = din("ix_pp", [64, 1], I32)                  # partner cc-partial gather

    o_e = dout("o_e", [128, HALF])                      # E_{s|q} half (channel-major)
    o_cc = dout("o_cc", [64, HALF])                     # cpam pixel half, my co-half

    with tile.TileContext(nc) as tc:
        with tc.tile_pool(name="per", bufs=1) as per, \
             tc.tile_pool(name="dram", bufs=1, space="DRAM") as dram:
            # ---- persistent tiles ----
            x1a = per.tile([128, HALF], F32R)
            x1b = per.tile([128, HALF], F32R)
            x2 = per.tile([128, HALF], F32R)
            vpm = per.tile([128, HALF], BF16)            # my v, pixel-major (u 0..15)
            vsec = per.tile([128, HALF], BF16)           # partner-half v (u 16..31)
            stem = per.tile([128, HALF], F32R)           # my stem output (= resid)
            gvec = per.tile([128, 1], F32)
            zslot = per.tile([128, NU], F32)
            etile = per.tile([128, HALF], F32)
            rbc = per.tile([128, HALF], F32)
            bneg = per.tile([128, 1], F32)
            nc.gpsimd.memset(bneg[:], EXP_BIAS)
            ones_u = per.tile([128, 1], BF16)
            nc.gpsimd.memset(ones_u[:], 1.0)

            ag2a_in = dram.tile([128, 2048], F32)
            ag2a_out = dram.tile([512, 2048], F32)
            ag2b_in = dram.tile([128, 2048], BF16)       # my v (pixel-major), bf16
            ag2b_out = dram.tile([256, 2048], BF16)      # 2-rank: [half0 v; half1 v]
            ag3_in = dram.tile([128, 32], F32)
            ag3_out = dram.tile([512, 32], F32)
            agh_in = dram.tile([128, 192], F32)          # halo: top row | bottom row | zeros
            agh_out = dram.tile([256, 192], F32)         # 2-rank same-role pair
            agp_in = dram.tile([64, 2048], BF16)         # partner's co-half partial
            agp_out = dram.tile([128, 2048], BF16)       # 2-rank cross-role pair
            dsc = dram.tile([16, 128], F32)             # transpose scratch for D row

            # ================= Phase A: stem + projections =================
            with nc.named_scope("pA_stem"), \
                 tc.tile_pool(name="pha", bufs=1) as pha, \
                 tc.tile_pool(name="psA", bufs=2, space="PSUM") as psA:
                # weights first so the first stem matmul isn't DMA-queued
                wstem = pha.tile([128, 2, 9, 128], F32R)
                nc.sync.dma_start(wstem[:], i_wstem[:].rearrange("a t p c -> p a t c").bitcast(F32R))
                bstem = pha.tile([128, 1], F32)
                nc.sync.dma_start(bstem[:], i_bstem[:])
                wkp = pha.tile([128, CH], F32R)
                nc.sync.dma_start(wkp[:], i_wkp[:].bitcast(F32R))
                bkp = pha.tile([CH, 1], F32)
                nc.sync.dma_start(bkp[:], i_bkp[:])
                wqp = pha.tile([128, CH], F32R)
                nc.sync.dma_start(wqp[:], i_wqp[:].bitcast(F32R))
                bqp = pha.tile([CH, 1], F32)
                nc.sync.dma_start(bqp[:], i_bqp[:])
                img = pha.tile([128, 2, NROW_IN * PW + 2], F32R)
                isrc = i_img[:].rearrange("a p x -> p a x").bitcast(F32R)
                for r0, r1 in ((0, 10), (10, 18), (18, 26), (26, 34)):
                    nc.sync.dma_start(img[:, :, r0 * PW:r1 * PW],
                                      isrc[:, :, r0 * PW:r1 * PW])

                kpart = pha.tile([CH, HALF], F32R)
                qpart = pha.tile([CH, HALF], F32R)
                for pc in range(NJ):                    # 4 pixel chunks of 512 (8 img rows)
                    sl = slice(pc * TC, (pc + 1) * TC)
                    ps = psA.tile([128, TC], F32)
                    first = True
                    for ch in range(2):
                        for t in range(9):
                            dy, dx = t // 3 - 1, t % 3 - 1
                            off = (8 * pc + dy + 1) * PW + (dx + 1)
                            rhs = img[:, ch, off:off + 8 * PW].rearrange(
                                "p (r w) -> p r w", r=8)[:, :, 0:64]
                            nc.tensor.matmul(ps[:], wstem[:, ch, t, :], rhs,
                                             start=first, stop=(ch == 1 and t == 8))
                            first = False
                    nc.scalar.activation(stem[:, sl], ps[:], AF.Relu, bias=bstem[:])
                    # fused per-chunk projections + collective payload
                    psk = psA.tile([CH, TC], F32)
                    nc.tensor.matmul(psk[:], wkp[:], stem[:, sl], start=True, stop=True)
                    nc.scalar.activation(kpart[:, sl], psk[:], AF.Identity, bias=bkp[:])
                    psq = psA.tile([CH, TC], F32)
                    nc.tensor.matmul(psq[:], wqp[:], stem[:, sl], start=True, stop=True)
                    nc.scalar.activation(qpart[:, sl], psq[:], AF.Identity, bias=bqp[:])
                    nc.sync.dma_start(ag2a_in[0:64, sl], kpart[:, sl].bitcast(F32))
                    nc.sync.dma_start(ag2a_in[64:128, sl], qpart[:, sl].bitcast(F32))

                with nc.named_scope("ag2a"):
                    nc.gpsimd.collective_compute(
                        "AllGather", ALU.bypass,
                        replica_groups=[[0, 1, 2, 3], [4, 5, 6, 7]],
                        ins=[ag2a_in.opt()], outs=[ag2a_out.opt()],
                    )

                # v (pixel-major, bf16) computed after the kq collective is
                # triggered; its 2-rank exchange rides behind ag2a on the
                # collective engine while scores run.
                wv = pha.tile([128, 128], F32)
                nc.sync.dma_start(wv[:], i_wv[:])
                bvrow = pha.tile([1, 128], F32)
                nc.sync.dma_start(bvrow[:], i_bvrow[:])
                bvbc = pha.tile([128, 128], F32)
                nc.gpsimd.partition_broadcast(bvbc[:], bvrow[:])
                for uc in range(16):
                    psv = psA.tile([128, 128], F32)
                    nc.tensor.matmul(psv[:], stem[:, uc * 128:(uc + 1) * 128].bitcast(F32),
                                     wv[:], start=True, stop=True)
                    nc.vector.tensor_tensor(vpm[:, uc * 128:(uc + 1) * 128],
                                            psv[:], bvbc[:], ALU.add)
                nc.sync.dma_start(ag2b_in[:], vpm[:])

                with nc.named_scope("ag2b"):
                    nc.gpsimd.collective_compute(
                        "AllGather", ALU.bypass,
                        replica_groups=[[0, 1], [2, 3], [4, 5], [6, 7]],
                        ins=[ag2b_in.opt()], outs=[ag2b_out.opt()],
                    )

            # ================= Phase B: gathers =================
            with nc.named_scope("pB_gather"), tc.tile_pool(name="phb", bufs=1) as phb:
                def gather(table, dst, idx_dram, cast=True):
                    idxt = phb.tile([128, 1], I32, name=f"idx_{dst.tensor.name}")
                    nc.sync.dma_start(idxt[:], idx_dram[:])
                    srcv = table[:].bitcast(F32R) if cast else table[:]
                    nc.gpsimd.indirect_dma_start(
                        out=dst[:], out_offset=None, in_=srcv,
                        in_offset=bass.IndirectOffsetOnAxis(ap=idxt[:, :1], axis=0))

                gather(ag2a_out, x1a, ix_x1a)
                gather(ag2a_out, x2, ix_x2)
                gather(ag2a_out, x1b, ix_x1b)

            # prefetch concat-conv weights during attention
            wcc = per.tile([128, 9, 128], F32R)
            nc.sync.dma_start(wcc[:], i_wcc[:].rearrange("t p c -> p t c").bitcast(F32R))
            bcc = per.tile([128, 1], F32)
            nc.sync.dma_start(bcc[:], i_bcc[:])

            # ================= Phase C: attention =================
            with nc.named_scope("pC_attn"), tc.tile_pool(name="phc", bufs=3) as phc, \
                 tc.tile_pool(name="psS", bufs=3, space="PSUM") as psS, \
                 tc.tile_pool(name="psO", bufs=1, space="PSUM") as psO:
                pv_ps = []
                for j in range(NJ):
                    pv_ps.append(psO.tile([128, TC], F32, name=f"pvps{j}"))
                ACT_ABS_U = set()      # ACT is the pace-setter now; abs all on DVE
                LAG = 6
                pts = {}
                deferred = []

                def emit_pv(uu):
                    vt = vpm if uu < 16 else vsec
                    uslv = slice((uu % 16) * 128, (uu % 16) * 128 + 128)
                    ptv = pts.pop(uu)
                    for j in range(NJ):
                        tsl = slice(j * TC, (j + 1) * TC)
                        nc.tensor.matmul(pv_ps[j][:], vt[:, uslv], ptv[:, tsl],
                                         start=(uu == 0), stop=(uu == NU - 1))

                for step in range(NU + LAG):
                    if step == 16:
                        # fetch the partner-half v here so no coalesced
                        # semaphore wait early in the loop can bind to it
                        idxv = phc.tile([128, 1], I32, name="idxv")
                        nc.sync.dma_start(idxv[:], ix_vp[:])
                        nc.gpsimd.indirect_dma_start(
                            out=vsec[:], out_offset=None, in_=ag2b_out[:],
                            in_offset=bass.IndirectOffsetOnAxis(ap=idxv[:, :1],
                                                                axis=0))
                    if step < NU:
                        u = step
                        x1t = x1a if u < 16 else x1b
                        usl = slice((u % 16) * 128, (u % 16) * 128 + 128)
                        pabs = phc.tile([128, HALF], F32, name="pabs", bufs=4)
                        for j2 in range(2):
                            t2 = slice(j2 * 2 * TC, (j2 * 2 + 2) * TC)
                            sps = psS.tile([128, 2 * TC], F32, name="sps", bufs=2)
                            for jj in range(2):
                                nc.tensor.matmul(
                                    sps[:, jj * TC:(jj + 1) * TC], x1t[:, usl],
                                    x2[:, (j2 * 2 + jj) * TC:(j2 * 2 + jj + 1) * TC],
                                    start=True, stop=True)
                            if u in ACT_ABS_U:
                                nc.scalar.activation(pabs[:, t2], sps[:], AF.Abs)
                            else:
                                nc.vector.tensor_scalar(pabs[:, t2].bitcast(I32),
                                                        sps[:].bitcast(I32),
                                                        0x7FFFFFFF, None, ALU.bitwise_and)
                        pt = phc.tile([128, HALF], BF16, name="p_t", bufs=LAG + 2)
                        nc.scalar.activation(pt[:], pabs[:], AF.Exp, bias=bneg[:],
                                             accum_out=zslot[:, u:u + 1])
                        pts[u] = pt
                    if step == NU:
                        # gate: sigmoid(w2 @ relu(w1 @ mean(v) + b1) + b2), mean folded in w1
                        w1t = phc.tile([128, CR], F32, name="w1t")
                        nc.sync.dma_start(w1t[:], i_w1t[:])
                        b1 = phc.tile([CR, 1], F32, name="b1")
                        nc.sync.dma_start(b1[:], i_b1[:])
                        w2t = phc.tile([CR, 128], F32, name="w2t")
                        nc.sync.dma_start(w2t[:], i_w2t[:])
                        b2 = phc.tile([128, 1], F32, name="b2")
                        nc.sync.dma_start(b2[:], i_b2[:])
                        psum_v = psS.tile([128, 1], F32, name="psum_v", tag="sps", bufs=2)
                        for half, vt in ((0, vpm), (1, vsec)):
                            for uc in range(16):
                                nc.tensor.matmul(psum_v[:],
                                                 vt[:, uc * 128:(uc + 1) * 128],
                                                 ones_u[:], start=(half == 0 and uc == 0),
                                                 stop=(half == 1 and uc == 15))
                        vsum = phc.tile([128, 1], F32, name="vsum")
                        nc.scalar.copy(vsum[:], psum_v[:])
                        psh = psS.tile([CR, 1], F32, name="psh", tag="sps", bufs=2)
                        nc.tensor.matmul(psh[:], w1t[:], vsum[:], start=True, stop=True)
                        hgate = phc.tile([CR, 1], F32, name="hgate")
                        nc.scalar.activation(hgate[:], psh[:], AF.Relu, bias=b1[:])
                        psg = psS.tile([128, 1], F32, name="psg", tag="sps", bufs=2)
                        nc.tensor.matmul(psg[:], w2t[:], hgate[:], start=True, stop=True)
                        nc.scalar.activation(gvec[:], psg[:], AF.Sigmoid, bias=b2[:])
                    if step >= LAG:
                        uu = step - LAG
                        # partner-half PVs (u>=16) are deferred so their
                        # weight-loads never block the in-order PE queue
                        # before the v exchange lands
                        if uu < 16:
                            emit_pv(uu)
                        else:
                            deferred.append(uu)
                        if step >= 22:
                            for _ in range(2):
                                if deferred:
                                    emit_pv(deferred.pop(0))
                while deferred:
                    emit_pv(deferred.pop(0))

                nc.sync.dma_start(ag3_in[:], zslot[:])

                with nc.named_scope("ag3"):
                    nc.gpsimd.collective_compute(
                        "AllGather", ALU.bypass,
                        replica_groups=[[0, 1, 2, 3], [4, 5, 6, 7]],
                        ins=[ag3_in.opt()], outs=[ag3_out.opt()],
                    )

                # D = sum of the two other-role partials, my half (fine rows of 16)
                with nc.named_scope("pD_efin"):
                    ag3f = ag3_out[:].rearrange("r (g w) -> (r g) w", w=16)
                    w1g = phc.tile([128, 16], F32, name="w1g")
                    w2g = phc.tile([128, 16], F32, name="w2g")
                    for dst, ixd in ((w1g, ix_d1), (w2g, ix_d2)):
                        idxt = phc.tile([128, 1], I32, name=f"ixd_{dst.tensor.name}")
                        nc.sync.dma_start(idxt[:], ixd[:])
                        nc.gpsimd.indirect_dma_start(
                            out=dst[:], out_offset=None, in_=ag3f,
                            in_offset=bass.IndirectOffsetOnAxis(ap=idxt[:, :1], axis=0))
                    dmine = phc.tile([128, 16], F32, name="dmine")
                    nc.vector.tensor_tensor(dmine[:], w1g[:], w2g[:], ALU.add)
                    rrec = phc.tile([128, 16], F32, name="rrec")
                    nc.vector.reciprocal(rrec[:], dmine[:])
                    nc.sync.dma_start(dsc[:].rearrange("c p -> p c"), rrec[:])
                    drow = phc.tile([1, HALF], F32, name="drow")
                    nc.sync.dma_start(drow[:], dsc[:].rearrange("c p -> (c p)").unsqueeze(0))
                    nc.gpsimd.partition_broadcast(rbc[:], drow[:])

                    # E = (PV * g) * R + resid; edge chunks first so the halo
                    # exchange can start while the interior finishes
                    halo = phc.tile([128, 192], F32, name="halo")
                    nc.vector.memset(halo[:, 128:192], 0.0)
                    for j in (0, 3, 1, 2):
                        tsl = slice(j * TC, (j + 1) * TC)
                        nc.vector.scalar_tensor_tensor(etile[:, tsl], pv_ps[j][:],
                                                       gvec[:], rbc[:, tsl],
                                                       ALU.mult, ALU.mult)
                        nc.vector.tensor_tensor(etile[:, tsl], etile[:, tsl],
                                                stem[:, tsl].bitcast(F32), ALU.add)
                        if j == 0:
                            nc.vector.tensor_copy(halo[:, 0:64], etile[:, 0:64])
                        elif j == 3:
                            nc.vector.tensor_copy(halo[:, 64:128],
                                                  etile[:, HALF - 64:HALF])
                            nc.sync.dma_start(agh_in[:], halo[:])
                    for j in range(NJ):
                        tsl = slice(j * TC, (j + 1) * TC)
                        nc.sync.dma_start(o_e[:, tsl], etile[:, tsl])

            with nc.named_scope("agh"):
                nc.gpsimd.collective_compute(
                    "AllGather", ALU.bypass,
                    replica_groups=[[0, 1], [2, 3], [4, 5], [6, 7]],
                    ins=[agh_in.opt()], outs=[agh_out.opt()],
                )

            # ============ Phase E: concat conv via per-role partials ============
            with nc.named_scope("pE_cc"), tc.tile_pool(name="phe", bufs=1) as phe, \
                 tc.tile_pool(name="psE", bufs=2, space="PSUM") as psE:
                EPW = NROW_IN * PW + 2                   # 34 padded rows of 66 (+slack)
                epadS = phe.tile([128, EPW], F32R)       # my E half, padded
                zsrc = phe.tile([128, PW], F32R)
                nc.vector.memset(zsrc[:].bitcast(F32), 0.0)
                # interior rows 1..32 <- my E (restride 64 -> 66)
                nc.sync.dma_start(
                    epadS[:, PW + 1:33 * PW + 1].rearrange("p (r w) -> p r w", w=PW)[:, :, 0:64],
                    etile[:].rearrange("p (r w) -> p r w", w=64).bitcast(F32R))
                # halo rows 0 and 33 via indexed gather from the halo table
                aghf = agh_out[:].rearrange("r (g w) -> (r g) w", w=64).bitcast(F32R)
                for ixd, row in ((ix_h1, 0), (ix_h2, 33)):
                    idxt = phe.tile([128, 1], I32, name=f"ixh_{row}")
                    nc.sync.dma_start(idxt[:], ixd[:])
                    nc.gpsimd.indirect_dma_start(
                        out=epadS[:, row * PW + 1:row * PW + 65], out_offset=None,
                        in_=aghf,
                        in_offset=bass.IndirectOffsetOnAxis(ap=idxt[:, :1], axis=0))
                # zero borders: left/right columns of all 34 rows
                lcol = epadS[:, 0:34 * PW].rearrange("p (r w) -> p r w", w=PW)[:, :, 0:1]
                nc.sync.dma_start(lcol, zsrc[:, 0:34].unsqueeze(2))
                rcol = epadS[:, 65:34 * PW - 1].rearrange("p (r w) -> p r w", w=PW)[:, :, 0:1]
                nc.sync.dma_start(rcol, zsrc[:, 0:33].unsqueeze(2))
                nc.sync.dma_start(epadS[:, 34 * PW - 1:34 * PW + 1], zsrc[:, 0:2])

                pcc = phe.tile([128, HALF], BF16)
                for pc in (1, 2, 0, 3):
                    ps = psE.tile([128, TC], F32)
                    for t in range(9):
                        dy, dx = t // 3 - 1, t % 3 - 1
                        off = (8 * pc + dy + 1) * PW + (dx + 1)
                        rhs = epadS[:, off:off + 8 * PW].rearrange(
                            "p (r w) -> p r w", r=8)[:, :, 0:64]
                        nc.tensor.matmul(ps[:], wcc[:, t, :], rhs,
                                         start=(t == 0), stop=(t == 8))
                    nc.scalar.copy(pcc[:, pc * TC:(pc + 1) * TC], ps[:])
                    nc.sync.dma_start(agp_in[:, pc * TC:(pc + 1) * TC],
                                      pcc[64:128, pc * TC:(pc + 1) * TC])

                with nc.named_scope("agp"):
                    nc.gpsimd.collective_compute(
                        "AllGather", ALU.bypass,
                        replica_groups=[[0, 2], [1, 3], [4, 6], [5, 7]],
                        ins=[agp_in.opt()], outs=[agp_out.opt()],
                    )
                ppart = phe.tile([64, HALF], BF16)
                idxp = phe.tile([64, 1], I32)
                nc.sync.dma_start(idxp[:], ix_pp[:])
                nc.gpsimd.indirect_dma_start(
                    out=ppart[:], out_offset=None, in_=agp_out[:],
                    in_offset=bass.IndirectOffsetOnAxis(ap=idxp[:, :1], axis=0))
                csum = phe.tile([64, HALF], BF16)
                nc.vector.tensor_tensor(csum[:], pcc[0:64, :], ppart[:], ALU.add)
                ccout = phe.tile([64, HALF], F32)
                nc.scalar.activation(ccout[:], csum[:], AF.Relu, bias=bcc[0:64, :])
                nc.sync.dma_start(o_cc[:], ccout[:])

    nc.compile()
    _cache['nc'] = nc
    return nc


# ====================== host-side preparation ======================

def _prep_inputs(inp):
    """Build the 8 per-core input dicts from the full problem inputs."""
    f32 = np.float32
    g = {k: np.asarray(v, f32) for k, v in inp.items()}
    eps = 1e-5

    def fold_stem(w, b, gam, be, m, v):
        s = gam / np.sqrt(v + eps)
        w_eff = w * s[:, None, None, None]                     # [co, cin, 3, 3]
        b_eff = (b - m) * s + be
        # [2, 9, 128, 128] : (cin chunk, tap, ci, co)
        wt = np.zeros((2, 9, 128, 128), f32)
        for ch in range(2):
            for t in range(9):
                wt[ch, t] = w_eff[:, ch * 128:(ch + 1) * 128, t // 3, t % 3].T
        return wt, b_eff.astype(f32).reshape(128, 1)

    ws_s, bs_s = fold_stem(g['ts_w'], g['ts_b'], g['ts_g'], g['ts_be'], g['ts_m'], g['ts_v'])
    ws_q, bs_q = fold_stem(g['tq_w'], g['tq_b'], g['tq_g'], g['tq_be'], g['tq_m'], g['tq_v'])

    s_cc = g['cc_g'] / np.sqrt(g['cc_v'] + eps)
    wcc_eff = g['cc_w'] * s_cc[:, None, None, None]     # [128, 256, 3, 3]
    bcc_eff = (g['cc_be'] - g['cc_m'] * s_cc).astype(f32)
    # role S convolves E_s (input channels 128:256); role Q convolves E_q
    # (0:128). Output channels are permuted so rows 0:64 are always the
    # co-half this core finalizes (S: 0:64, Q: 64:128) and rows 64:128 the
    # half it ships to its cross-role partner.
    co_perm = {'s': np.arange(128),
               'q': np.concatenate([np.arange(64, 128), np.arange(0, 64)])}
    wcc_role = {}
    bcc_role = {}
    for role, c0 in (('s', 128), ('q', 0)):
        perm = co_perm[role]
        wt = np.zeros((9, 128, 128), f32)
        for t in range(9):
            wt[t] = wcc_eff[perm][:, c0:c0 + 128, t // 3, t % 3].T
        wcc_role[role] = wt
        bcc_role[role] = bcc_eff[perm].reshape(128, 1)

    wv = np.ascontiguousarray(g['cv_w'][:, :, 0, 0].T)         # [ci, c]
    bvrow = g['cv_b'].reshape(1, 128)
    wk1 = np.ascontiguousarray(g['k1_w'][:, :, 0, 0].T)        # [ci, 64]
    wk2n = np.ascontiguousarray((-g['k2_w'][:, :, 0, 0]).T)
    wq1 = np.ascontiguousarray(g['q1_w'][:, :, 0, 0].T)
    wq2 = np.ascontiguousarray(g['q2_w'][:, :, 0, 0].T)
    w1t = np.ascontiguousarray(g['g1_w'].T) / float(N)         # [128, 8] mean-folded
    b1 = g['g1_b'].reshape(CR, 1)
    w2t = np.ascontiguousarray(g['g2_w'].T)                    # [8, 128]
    b2 = g['g2_b'].reshape(128, 1)

    def pad_img(x, h):                                          # x [256, 64, 64]
        out = np.zeros((256, NROW_IN, PW), f32)
        r0, r1 = 32 * h - 1, 32 * h + 33
        cr0, cr1 = max(r0, 0), min(r1, H)
        out[:, cr0 - r0:cr1 - r0, 1:65] = x[:, cr0:cr1, :]
        return out.reshape(2, 128, NROW_IN * PW)

    P = np.arange(128)
    def rows(sec, slot, chan_off=0):
        # AG2a table: 128-row sections [k:0, q:64]; AG2b: 256-row sections [v:0, st:128]
        if slot in ('k', 'q'):
            base = 128 * sec + (0 if slot == 'k' else 64)
        else:
            base = 256 * sec + (0 if slot == 'v' else 128)
        return (base + chan_off + P).astype(np.int32).reshape(128, 1)

    def rows64(sec_lo, slot, sec_hi):
        lo = rows(sec_lo, slot)[0:64, 0]
        hi = rows(sec_hi, slot)[0:64, 0]
        return np.concatenate([lo, hi]).astype(np.int32).reshape(128, 1)

    in_maps = []
    for c in range(8):
        b, r = c // 4, c % 4
        role_s = r < 2
        h = r % 2
        img_full = g['s'][b] if role_s else g['q'][b]
        d = {
            'img': pad_img(img_full, h),
            'wstem': ws_s if role_s else ws_q,
            'bstem': bs_s if role_s else bs_q,
            'wkp': wk1 if role_s else wk2n,
            'bkp': (g['k1_b'] if role_s else -g['k2_b']).reshape(CH, 1),
            'wqp': wq1 if role_s else wq2,
            'bqp': (g['q1_b'] if role_s else g['q2_b']).reshape(CH, 1),
            'wv': wv, 'bvrow': bvrow,
            'w1t': w1t, 'b1': b1, 'w2t': w2t, 'b2': b2,
        }
        d['wcc'] = wcc_role['s' if role_s else 'q']
        d['bcc'] = bcc_role['s' if role_s else 'q']
        # gather indices (sections: 0,1 = s-img halves; 2,3 = q-img halves).
        # u-chunk order is LOCAL-half-first: x1a covers my own pixel half so
        # PV steps 0..15 can run on the locally computed v before the v
        # exchange lands.
        slot1 = 'q' if role_s else 'k'
        d['ix_x1a'] = rows64(h, slot1, 2 + h)
        d['ix_x1b'] = rows64(1 - h, slot1, 2 + (1 - h))
        d['ix_x2'] = rows64(h, 'k' if role_s else 'q', 2 + h)
        osecs = (2, 3) if role_s else (0, 1)
        # z-partial granule of section `osec` that covers MY pixels: granule 0
        # holds osec's own half (its local-first order), granule 1 the other.
        d['ix_d1'] = ((128 * osecs[0] + P) * 2 + h).astype(np.int32).reshape(128, 1)
        d['ix_d2'] = ((128 * osecs[1] + P) * 2 + (1 - h)).astype(np.int32).reshape(128, 1)
        # partner v rows in the 2-rank ag2b table (my rank within pair = h)
        d['ix_vp'] = (128 * (1 - h) + P).astype(np.int32).reshape(128, 1)
        # halo gathers from the 2-rank halo table ([256 rows, 3 granules of
        # 64]): rank within the same-role pair = h; granule 0 = top row of
        # that core's E half, 1 = bottom row, 2 = zeros
        myrk, prk = h, 1 - h
        if h == 0:
            top = (128 * myrk + P) * 3 + 2           # zeros (image row -1)
            bot = (128 * prk + P) * 3 + 0            # partner's first row (row 32)
        else:
            top = (128 * prk + P) * 3 + 1            # partner's last row (row 31)
            bot = (128 * myrk + P) * 3 + 2           # zeros (image row 64)
        d['ix_h1'] = top.astype(np.int32).reshape(128, 1)
        d['ix_h2'] = bot.astype(np.int32).reshape(128, 1)
        # cc-partial partner = other rank of the 2-rank cross-role pair
        pprk = 1 - (0 if r < 2 else 1)
        d['ix_pp'] = (64 * pprk + np.arange(64)).astype(np.int32).reshape(64, 1)
        in_maps.append(d)
    return in_maps


def _assemble(results):
    cpam = np.zeros((B, C, H, W), np.float32)
    e_q = np.zeros((B, C, H, W), np.float32)
    e_s = np.zeros((B, C, H, W), np.float32)
    for c in range(8):
        b, r = c // 4, c % 4
        h = r % 2
        e_half = results[c]['o_e'].reshape(C, 32, W)
        tgt = e_s if r < 2 else e_q
        tgt[b, :, 32 * h:32 * h + 32, :] = e_half
        co0 = 0 if r < 2 else 64
        cpam[b, co0:co0 + 64, 32 * h:32 * h + 32, :] = \
            results[c]['o_cc'].reshape(64, 32, W)
    return cpam, e_q, e_s


def kernel(**inputs):
    nc = build_program()
    in_maps = _prep_inputs(inputs)
    res = bass_utils.run_bass_kernel_spmd(nc, in_maps, core_ids=list(range(8)))
    return _assemble(res.results)


def kernel_traced(**inputs):
    """Like kernel() but reporting a time estimate.

    Tries NTFF tracing (real HW exec time); if the profiling hook is not
    available in this environment, falls back to the Tile cost-model
    timeline simulation (single-core makespan; its collective cost model
    assumes cross-chip scale, so it substantially over-estimates the
    intra-chip AllGathers this kernel uses).
    """
    nc = build_program()
    in_maps = _prep_inputs(inputs)
    exec_ns = None
    try:
        res = bass_utils.run_bass_kernel_spmd(nc, in_maps, core_ids=list(range(8)),
                                              trace=True)
        exec_ns = res.exec_time_ns
    except Exception:
        res = bass_utils.run_bass_kernel_spmd(nc, in_maps, core_ids=list(range(8)))
    if exec_ns is None:
        try:
            from concourse.timeline_sim import TimelineSim
            exec_ns = int(TimelineSim(nc, no_exec=True, trace=False).simulate())
        except Exception:
            exec_ns = -1
    return _assemble(res.results), exec_ns



# revision 55
# speedup vs baseline: 1.1633x; 1.0044x over previous
"""Trainium2 Bass kernel for nn_FEM_best (dual-attention fusion module).

Decomposition over 8 NeuronCores: core c = b*4 + r, b in {0,1}, r in:
  r=0: role S (computes E_s), pixel half 0   (phase-1: s-image stem, half 0)
  r=1: role S, half 1                        (phase-1: s-image stem, half 1)
  r=2: role Q (computes E_q), half 0         (phase-1: q-image stem, half 0)
  r=3: role Q, half 1                        (phase-1: q-image stem, half 1)
plus a channel-quarter of the final concat-conv on every core.

Single SPMD program; all per-core asymmetry is carried by input data
(weights, pre-padded image halves, and row-gather index tensors).
Cross-core exchange via AllGather collectives through DRAM bounce
tables + indirect row-gather DMAs.
"""
import sys, os
sys.path.insert(0, '/opt/trn_rl_repo')
import numpy as np

import concourse.bass as bass
import concourse.mybir as mybir
import concourse.bacc as bacc
import concourse.tile as tile
from concourse import bass_utils

F32 = mybir.dt.float32
F32R = mybir.dt.float32r
I32 = mybir.dt.int32
AF = mybir.ActivationFunctionType
ALU = mybir.AluOpType

B, CIN, H, W = 2, 256, 64, 64
N = H * W                 # 4096
C = 128                   # inter channels
CH = 64                   # C//2 (k/q projection channels)
CR = 8                    # gate bottleneck
HALF = N // 2             # 2048
PW = W + 2                # padded row width 66
NROW_IN = 34              # input rows per half (32 + 2 halo)
TC = 512                  # tile free size
NJ = HALF // TC           # 4 t-chunks per half
NU = N // 128             # 32 u-chunks
EXP_BIAS = -40.0
BF16 = mybir.dt.bfloat16

_cache = {}


def build_program():
    if 'nc' in _cache:
        return _cache['nc']
    nc = bacc.Bacc("TRN2", target_bir_lowering=False, debug=False, num_devices=8)

    def din(name, shape, dt=F32):
        return nc.dram_tensor(name, list(shape), dt, kind="ExternalInput").ap()

    def dout(name, shape, dt=F32):
        return nc.dram_tensor(name, list(shape), dt, kind="ExternalOutput").ap()

    i_img = din("img", [2, 128, NROW_IN * PW])          # padded input image half, 2 cin chunks
    i_wstem = din("wstem", [2, 9, 128, 128])            # (chunk, tap, ci, co) BN-folded
    i_bstem = din("bstem", [128, 1])
    i_wkp = din("wkp", [128, CH])                       # k-part projection (sign-folded)
    i_bkp = din("bkp", [CH, 1])
    i_wqp = din("wqp", [128, CH])
    i_bqp = din("bqp", [CH, 1])
    i_wv = din("wv", [128, 128])                        # cv_w^T [ci, c]
    i_bvrow = din("bvrow", [1, 128])                    # cv_b as a row
    i_wcc = din("wcc", [9, 128, 128])                   # cc conv, my role's cin half
    i_bcc = din("bcc", [128, 1])
    i_w1t = din("w1t", [128, CR])                       # gate MLP (mean-folded)
    i_b1 = din("b1", [CR, 1])
    i_w2t = din("w2t", [CR, 128])
    i_b2 = din("b2", [128, 1])
    ix_x1a = din("ix_x1a", [128, 1], I32)               # row-gather indices into AG2 table
    ix_x1b = din("ix_x1b", [128, 1], I32)
    ix_x2 = din("ix_x2", [128, 1], I32)
    ix_vp = din("ix_vp", [128, 1], I32)                 # partner v rows in ag2b table
    ix_d1 = din("ix_d1", [128, 1], I32)                 # fine-row indices into AG3 table
    ix_d2 = din("ix_d2", [128, 1], I32)
    ix_h1 = din("ix_h1", [128, 1], I32)                 # top-halo row gather
    ix_h2 = din("ix_h2", [128, 1], I32)                 # bottom-halo row gather
    ix_pp = din("ix_pp", [64, 1], I32)                  # partner cc-partial gather

    o_e = dout("o_e", [128, HALF])                      # E_{s|q} half (channel-major)
    o_cc = dout("o_cc", [64, HALF])                     # cpam pixel half, my co-half

    with tile.TileContext(nc) as tc:
        with tc.tile_pool(name="per", bufs=1) as per, \
             tc.tile_pool(name="dram", bufs=1, space="DRAM") as dram:
            # ---- persistent tiles ----
            x1a = per.tile([128, HALF], F32R)
            x1b = per.tile([128, HALF], F32R)
            x2 = per.tile([128, HALF], F32R)
            vpm = per.tile([128, HALF], BF16)            # my v, pixel-major (u 0..15)
            vsec = per.tile([128, HALF], BF16)           # partner-half v (u 16..31)
            stem = per.tile([128, HALF], F32R)           # my stem output (= resid)
            gvec = per.tile([128, 1], F32)
            zslot = per.tile([128, NU], F32)
            etile = per.tile([128, HALF], F32)
            rbc = per.tile([128, HALF], F32)
            bneg = per.tile([128, 1], F32)
            nc.gpsimd.memset(bneg[:], EXP_BIAS)
            ones_u = per.tile([128, 1], BF16)
            nc.gpsimd.memset(ones_u[:], 1.0)

            ag2a_in = dram.tile([128, 2048], F32)
            ag2a_out = dram.tile([512, 2048], F32)
            ag2b_in = dram.tile([128, 2048], BF16)       # my v (pixel-major), bf16
            ag2b_out = dram.tile([256, 2048], BF16)      # 2-rank: [half0 v; half1 v]
            ag3_in = dram.tile([128, 32], F32)
            ag3_out = dram.tile([512, 32], F32)
            agh_in = dram.tile([128, 192], F32)          # halo: top row | bottom row | zeros
            agh_out = dram.tile([256, 192], F32)         # 2-rank same-role pair
            agp_in = dram.tile([64, 2048], BF16)         # partner's co-half partial
            agp_out = dram.tile([128, 2048], BF16)       # 2-rank cross-role pair
            dsc = dram.tile([16, 128], F32)             # transpose scratch for D row

            # ================= Phase A: stem + projections =================
            with nc.named_scope("pA_stem"), \
                 tc.tile_pool(name="pha", bufs=1) as pha, \
                 tc.tile_pool(name="psA", bufs=2, space="PSUM") as psA:
                # weights first so the first stem matmul isn't DMA-queued
                wstem = pha.tile([128, 2, 9, 128], F32R)
                nc.sync.dma_start(wstem[:], i_wstem[:].rearrange("a t p c -> p a t c").bitcast(F32R))
                bstem = pha.tile([128, 1], F32)
                nc.sync.dma_start(bstem[:], i_bstem[:])
                wkp = pha.tile([128, CH], F32R)
                nc.sync.dma_start(wkp[:], i_wkp[:].bitcast(F32R))
                bkp = pha.tile([CH, 1], F32)
                nc.sync.dma_start(bkp[:], i_bkp[:])
                wqp = pha.tile([128, CH], F32R)
                nc.sync.dma_start(wqp[:], i_wqp[:].bitcast(F32R))
                bqp = pha.tile([CH, 1], F32)
                nc.sync.dma_start(bqp[:], i_bqp[:])
                img = pha.tile([128, 2, NROW_IN * PW + 2], F32R)
                isrc = i_img[:].rearrange("a p x -> p a x").bitcast(F32R)
                for r0, r1 in ((0, 10), (10, 18), (18, 26), (26, 34)):
                    nc.sync.dma_start(img[:, :, r0 * PW:r1 * PW],
                                      isrc[:, :, r0 * PW:r1 * PW])

                # warm-up operand for the PE pstate ramp
                wup = pha.tile([128, TC], F32)
                nc.vector.memset(wup[:], 0.0)

                kpart = pha.tile([CH, HALF], F32R)
                qpart = pha.tile([CH, HALF], F32R)
                for pc in range(NJ):                    # 4 pixel chunks of 512 (8 img rows)
                    sl = slice(pc * TC, (pc + 1) * TC)
                    ps = psA.tile([128, TC], F32)
                    if pc == 0:
                        # warm the PE ramp into this bank while the image
                        # DMA lands; start=True on the real chain clears it
                        for _ in range(8):
                            nc.tensor.matmul(ps[:], wup[:, 0:128], wup[:],
                                             start=True, stop=True)
                    first = True
                    for ch in range(2):
                        for t in range(9):
                            dy, dx = t // 3 - 1, t % 3 - 1
                            off = (8 * pc + dy + 1) * PW + (dx + 1)
                            rhs = img[:, ch, off:off + 8 * PW].rearrange(
                                "p (r w) -> p r w", r=8)[:, :, 0:64]
                            nc.tensor.matmul(ps[:], wstem[:, ch, t, :], rhs,
                                             start=first, stop=(ch == 1 and t == 8))
                            first = False
                    nc.scalar.activation(stem[:, sl], ps[:], AF.Relu, bias=bstem[:])
                    # fused per-chunk projections + collective payload
                    psk = psA.tile([CH, TC], F32)
                    nc.tensor.matmul(psk[:], wkp[:], stem[:, sl], start=True, stop=True)
                    nc.scalar.activation(kpart[:, sl], psk[:], AF.Identity, bias=bkp[:])
                    psq = psA.tile([CH, TC], F32)
                    nc.tensor.matmul(psq[:], wqp[:], stem[:, sl], start=True, stop=True)
                    nc.scalar.activation(qpart[:, sl], psq[:], AF.Identity, bias=bqp[:])
                    nc.sync.dma_start(ag2a_in[0:64, sl], kpart[:, sl].bitcast(F32))
                    nc.sync.dma_start(ag2a_in[64:128, sl], qpart[:, sl].bitcast(F32))

                with nc.named_scope("ag2a"):
                    nc.gpsimd.collective_compute(
                        "AllGather", ALU.bypass,
                        replica_groups=[[0, 1, 2, 3], [4, 5, 6, 7]],
                        ins=[ag2a_in.opt()], outs=[ag2a_out.opt()],
                    )

                # v (pixel-major, bf16) computed after the kq collective is
                # triggered; its 2-rank exchange rides behind ag2a on the
                # collective engine while scores run.
                wv = pha.tile([128, 128], F32)
                nc.sync.dma_start(wv[:], i_wv[:])
                bvrow = pha.tile([1, 128], F32)
                nc.sync.dma_start(bvrow[:], i_bvrow[:])
                bvbc = pha.tile([128, 128], F32)
                nc.gpsimd.partition_broadcast(bvbc[:], bvrow[:])
                for uc in range(16):
                    psv = psA.tile([128, 128], F32)
                    nc.tensor.matmul(psv[:], stem[:, uc * 128:(uc + 1) * 128].bitcast(F32),
                                     wv[:], start=True, stop=True)
                    nc.vector.tensor_tensor(vpm[:, uc * 128:(uc + 1) * 128],
                                            psv[:], bvbc[:], ALU.add)
                nc.sync.dma_start(ag2b_in[:], vpm[:])

                with nc.named_scope("ag2b"):
                    nc.gpsimd.collective_compute(
                        "AllGather", ALU.bypass,
                        replica_groups=[[0, 1], [2, 3], [4, 5], [6, 7]],
                        ins=[ag2b_in.opt()], outs=[ag2b_out.opt()],
                    )

            # ================= Phase B: gathers =================
            with nc.named_scope("pB_gather"), tc.tile_pool(name="phb", bufs=1) as phb:
                def gather(table, dst, idx_dram, cast=True):
                    idxt = phb.tile([128, 1], I32, name=f"idx_{dst.tensor.name}")
                    nc.sync.dma_start(idxt[:], idx_dram[:])
                    srcv = table[:].bitcast(F32R) if cast else table[:]
                    nc.gpsimd.indirect_dma_start(
                        out=dst[:], out_offset=None, in_=srcv,
                        in_offset=bass.IndirectOffsetOnAxis(ap=idxt[:, :1], axis=0))

                gather(ag2a_out, x1a, ix_x1a)
                gather(ag2a_out, x2, ix_x2)
                gather(ag2a_out, x1b, ix_x1b)

            # prefetch index tiles used after attention (keeps them off the
            # post-attention critical path)
            ixd1t = per.tile([128, 1], I32)
            nc.sync.dma_start(ixd1t[:], ix_d1[:])
            ixd2t = per.tile([128, 1], I32)
            nc.sync.dma_start(ixd2t[:], ix_d2[:])
            ixh1t = per.tile([128, 1], I32)
            nc.sync.dma_start(ixh1t[:], ix_h1[:])
            ixh2t = per.tile([128, 1], I32)
            nc.sync.dma_start(ixh2t[:], ix_h2[:])
            ixppt = per.tile([64, 1], I32)
            nc.sync.dma_start(ixppt[:], ix_pp[:])

            # prefetch concat-conv weights during attention
            wcc = per.tile([128, 9, 128], F32R)
            nc.sync.dma_start(wcc[:], i_wcc[:].rearrange("t p c -> p t c").bitcast(F32R))
            bcc = per.tile([128, 1], F32)
            nc.sync.dma_start(bcc[:], i_bcc[:])

            # ================= Phase C: attention =================
            with nc.named_scope("pC_attn"), tc.tile_pool(name="phc", bufs=3) as phc, \
                 tc.tile_pool(name="psS", bufs=3, space="PSUM") as psS, \
                 tc.tile_pool(name="psO", bufs=1, space="PSUM") as psO:
                pv_ps = []
                for j in range(NJ):
                    pv_ps.append(psO.tile([128, TC], F32, name=f"pvps{j}"))
                ACT_ABS_U = set()      # ACT is the pace-setter now; abs all on DVE
                LAG = 6
                pts = {}
                deferred = []

                def emit_pv(uu):
                    vt = vpm if uu < 16 else vsec
                    uslv = slice((uu % 16) * 128, (uu % 16) * 128 + 128)
                    ptv = pts.pop(uu)
                    for j in range(NJ):
                        tsl = slice(j * TC, (j + 1) * TC)
                        nc.tensor.matmul(pv_ps[j][:], vt[:, uslv], ptv[:, tsl],
                                         start=(uu == 0), stop=(uu == NU - 1))

                for step in range(NU + LAG):
                    if step == 16:
                        # fetch the partner-half v here so no coalesced
                        # semaphore wait early in the loop can bind to it
                        idxv = phc.tile([128, 1], I32, name="idxv")
                        nc.sync.dma_start(idxv[:], ix_vp[:])
                        nc.gpsimd.indirect_dma_start(
                            out=vsec[:], out_offset=None, in_=ag2b_out[:],
                            in_offset=bass.IndirectOffsetOnAxis(ap=idxv[:, :1],
                                                                axis=0))
                    if step < NU:
                        u = step
                        x1t = x1a if u < 16 else x1b
                        usl = slice((u % 16) * 128, (u % 16) * 128 + 128)
                        pabs = phc.tile([128, HALF], F32, name="pabs", bufs=4)
                        for j2 in range(2):
                            t2 = slice(j2 * 2 * TC, (j2 * 2 + 2) * TC)
                            sps = psS.tile([128, 2 * TC], F32, name="sps", bufs=2)
                            for jj in range(2):
                                nc.tensor.matmul(
                                    sps[:, jj * TC:(jj + 1) * TC], x1t[:, usl],
                                    x2[:, (j2 * 2 + jj) * TC:(j2 * 2 + jj + 1) * TC],
                                    start=True, stop=True)
                            if u in ACT_ABS_U:
                                nc.scalar.activation(pabs[:, t2], sps[:], AF.Abs)
                            else:
                                nc.vector.tensor_scalar(pabs[:, t2].bitcast(I32),
                                                        sps[:].bitcast(I32),
                                                        0x7FFFFFFF, None, ALU.bitwise_and)
                        pt = phc.tile([128, HALF], BF16, name="p_t", bufs=LAG + 2)
                        nc.scalar.activation(pt[:], pabs[:], AF.Exp, bias=bneg[:],
                                             accum_out=zslot[:, u:u + 1])
                        pts[u] = pt
                    if step == NU:
                        # gate: sigmoid(w2 @ relu(w1 @ mean(v) + b1) + b2), mean folded in w1
                        w1t = phc.tile([128, CR], F32, name="w1t")
                        nc.sync.dma_start(w1t[:], i_w1t[:])
                        b1 = phc.tile([CR, 1], F32, name="b1")
                        nc.sync.dma_start(b1[:], i_b1[:])
                        w2t = phc.tile([CR, 128], F32, name="w2t")
                        nc.sync.dma_start(w2t[:], i_w2t[:])
                        b2 = phc.tile([128, 1], F32, name="b2")
                        nc.sync.dma_start(b2[:], i_b2[:])
                        psum_v = psS.tile([128, 1], F32, name="psum_v", tag="sps", bufs=2)
                        for half, vt in ((0, vpm), (1, vsec)):
                            for uc in range(16):
                                nc.tensor.matmul(psum_v[:],
                                                 vt[:, uc * 128:(uc + 1) * 128],
                                                 ones_u[:], start=(half == 0 and uc == 0),
                                                 stop=(half == 1 and uc == 15))
                        vsum = phc.tile([128, 1], F32, name="vsum")
                        nc.scalar.copy(vsum[:], psum_v[:])
                        psh = psS.tile([CR, 1], F32, name="psh", tag="sps", bufs=2)
                        nc.tensor.matmul(psh[:], w1t[:], vsum[:], start=True, stop=True)
                        hgate = phc.tile([CR, 1], F32, name="hgate")
                        nc.scalar.activation(hgate[:], psh[:], AF.Relu, bias=b1[:])
                        psg = psS.tile([128, 1], F32, name="psg", tag="sps", bufs=2)
                        nc.tensor.matmul(psg[:], w2t[:], hgate[:], start=True, stop=True)
                        nc.scalar.activation(gvec[:], psg[:], AF.Sigmoid, bias=b2[:])
                    if step >= LAG:
                        uu = step - LAG
                        # partner-half PVs (u>=16) are deferred so their
                        # weight-loads never block the in-order PE queue
                        # before the v exchange lands
                        if uu < 16:
                            emit_pv(uu)
                        else:
                            deferred.append(uu)
                        if step >= 22:
                            for _ in range(2):
                                if deferred:
                                    emit_pv(deferred.pop(0))
                while deferred:
                    emit_pv(deferred.pop(0))

                nc.sync.dma_start(ag3_in[:], zslot[:])

                with nc.named_scope("ag3"):
                    nc.gpsimd.collective_compute(
                        "AllGather", ALU.bypass,
                        replica_groups=[[0, 1, 2, 3], [4, 5, 6, 7]],
                        ins=[ag3_in.opt()], outs=[ag3_out.opt()],
                    )

                # D = sum of the two other-role partials, my half (fine rows of 16)
                with nc.named_scope("pD_efin"):
                    ag3f = ag3_out[:].rearrange("r (g w) -> (r g) w", w=16)
                    w1g = phc.tile([128, 16], F32, name="w1g")
                    w2g = phc.tile([128, 16], F32, name="w2g")
                    for dst, idxt in ((w1g, ixd1t), (w2g, ixd2t)):
                        nc.gpsimd.indirect_dma_start(
                            out=dst[:], out_offset=None, in_=ag3f,
                            in_offset=bass.IndirectOffsetOnAxis(ap=idxt[:, :1], axis=0))
                    dmine = phc.tile([128, 16], F32, name="dmine")
                    nc.vector.tensor_tensor(dmine[:], w1g[:], w2g[:], ALU.add)
                    rrec = phc.tile([128, 16], F32, name="rrec")
                    nc.vector.reciprocal(rrec[:], dmine[:])
                    nc.sync.dma_start(dsc[:].rearrange("c p -> p c"), rrec[:])
                    drow = phc.tile([1, HALF], F32, name="drow")
                    nc.sync.dma_start(drow[:], dsc[:].rearrange("c p -> (c p)").unsqueeze(0))
                    nc.gpsimd.partition_broadcast(rbc[:], drow[:])

                    # E = (PV * g) * R + resid; edge chunks first so the halo
                    # exchange can start while the interior finishes
                    halo = phc.tile([128, 192], F32, name="halo")
                    nc.vector.memset(halo[:, 128:192], 0.0)
                    for j in (0, 3, 1, 2):
                        tsl = slice(j * TC, (j + 1) * TC)
                        nc.vector.scalar_tensor_tensor(etile[:, tsl], pv_ps[j][:],
                                                       gvec[:], rbc[:, tsl],
                                                       ALU.mult, ALU.mult)
                        nc.vector.tensor_tensor(etile[:, tsl], etile[:, tsl],
                                                stem[:, tsl].bitcast(F32), ALU.add)
                        if j == 0:
                            nc.vector.tensor_copy(halo[:, 0:64], etile[:, 0:64])
                        elif j == 3:
                            nc.vector.tensor_copy(halo[:, 64:128],
                                                  etile[:, HALF - 64:HALF])
                            nc.sync.dma_start(agh_in[:], halo[:])


            with nc.named_scope("agh"):
                nc.gpsimd.collective_compute(
                    "AllGather", ALU.bypass,
                    replica_groups=[[0, 1], [2, 3], [4, 5], [6, 7]],
                    ins=[agh_in.opt()], outs=[agh_out.opt()],
                )

            # ============ Phase E: concat conv via per-role partials ============
            with nc.named_scope("pE_cc"), tc.tile_pool(name="phe", bufs=1) as phe, \
                 tc.tile_pool(name="psE", bufs=2, space="PSUM") as psE:
                EPW = NROW_IN * PW + 2                   # 34 padded rows of 66 (+slack)
                epadS = phe.tile([128, EPW], F32R)       # my E half, padded
                zsrc = phe.tile([128, PW], F32R)
                nc.vector.memset(zsrc[:].bitcast(F32), 0.0)
                # interior rows 1..32 <- my E (restride 64 -> 66)
                nc.sync.dma_start(
                    epadS[:, PW + 1:33 * PW + 1].rearrange("p (r w) -> p r w", w=PW)[:, :, 0:64],
                    etile[:].rearrange("p (r w) -> p r w", w=64).bitcast(F32R))
                # halo rows 0 and 33 via indexed gather from the halo table
                aghf = agh_out[:].rearrange("r (g w) -> (r g) w", w=64).bitcast(F32R)
                for idxt, row in ((ixh1t, 0), (ixh2t, 33)):
                    nc.gpsimd.indirect_dma_start(
                        out=epadS[:, row * PW + 1:row * PW + 65], out_offset=None,
                        in_=aghf,
                        in_offset=bass.IndirectOffsetOnAxis(ap=idxt[:, :1], axis=0))
                # zero borders: left/right columns of all 34 rows
                lcol = epadS[:, 0:34 * PW].rearrange("p (r w) -> p r w", w=PW)[:, :, 0:1]
                nc.sync.dma_start(lcol, zsrc[:, 0:34].unsqueeze(2))
                rcol = epadS[:, 65:34 * PW - 1].rearrange("p (r w) -> p r w", w=PW)[:, :, 0:1]
                nc.sync.dma_start(rcol, zsrc[:, 0:33].unsqueeze(2))
                nc.sync.dma_start(epadS[:, 34 * PW - 1:34 * PW + 1], zsrc[:, 0:2])

                # keep the PE ramp warm through the post-attention idle
                wps2 = psE.tile([128, TC], F32, name="wps2")
                for _ in range(40):
                    nc.tensor.matmul(wps2[:], stem[:, 0:128].bitcast(F32),
                                     stem[:, 0:TC].bitcast(F32),
                                     start=True, stop=True)
                pcc = phe.tile([128, HALF], BF16)
                for pc in (1, 2, 0, 3):
                    ps = psE.tile([128, TC], F32)
                    for t in range(9):
                        dy, dx = t // 3 - 1, t % 3 - 1
                        off = (8 * pc + dy + 1) * PW + (dx + 1)
                        rhs = epadS[:, off:off + 8 * PW].rearrange(
                            "p (r w) -> p r w", r=8)[:, :, 0:64]
                        nc.tensor.matmul(ps[:], wcc[:, t, :], rhs,
                                         start=(t == 0), stop=(t == 8))
                    nc.scalar.copy(pcc[:, pc * TC:(pc + 1) * TC], ps[:])
                    nc.sync.dma_start(agp_in[:, pc * TC:(pc + 1) * TC],
                                      pcc[64:128, pc * TC:(pc + 1) * TC])

                with nc.named_scope("agp"):
                    nc.gpsimd.collective_compute(
                        "AllGather", ALU.bypass,
                        replica_groups=[[0, 2], [1, 3], [4, 6], [5, 7]],
                        ins=[agp_in.opt()], outs=[agp_out.opt()],
                    )
                ppart = phe.tile([64, HALF], BF16)
                csum = phe.tile([64, HALF], BF16)
                ccout = phe.tile([64, HALF], F32)
                nc.gpsimd.indirect_dma_start(
                    out=ppart[:], out_offset=None, in_=agp_out[:],
                    in_offset=bass.IndirectOffsetOnAxis(ap=ixppt[:, :1], axis=0))
                nc.vector.tensor_tensor(csum[:], pcc[0:64, :], ppart[:], ALU.add)
                nc.scalar.activation(ccout[:], csum[:], AF.Relu, bias=bcc[0:64, :])
                nc.sync.dma_start(o_cc[:], ccout[:])
                for j in range(NJ):
                    tsl = slice(j * TC, (j + 1) * TC)
                    nc.sync.dma_start(o_e[:, tsl], etile[:, tsl])

    nc.compile()
    _cache['nc'] = nc
    return nc


# ====================== host-side preparation ======================

def _prep_inputs(inp):
    """Build the 8 per-core input dicts from the full problem inputs."""
    f32 = np.float32
    g = {k: np.asarray(v, f32) for k, v in inp.items()}
    eps = 1e-5

    def fold_stem(w, b, gam, be, m, v):
        s = gam / np.sqrt(v + eps)
        w_eff = w * s[:, None, None, None]                     # [co, cin, 3, 3]
        b_eff = (b - m) * s + be
        # [2, 9, 128, 128] : (cin chunk, tap, ci, co)
        wt = np.zeros((2, 9, 128, 128), f32)
        for ch in range(2):
            for t in range(9):
                wt[ch, t] = w_eff[:, ch * 128:(ch + 1) * 128, t // 3, t % 3].T
        return wt, b_eff.astype(f32).reshape(128, 1)

    ws_s, bs_s = fold_stem(g['ts_w'], g['ts_b'], g['ts_g'], g['ts_be'], g['ts_m'], g['ts_v'])
    ws_q, bs_q = fold_stem(g['tq_w'], g['tq_b'], g['tq_g'], g['tq_be'], g['tq_m'], g['tq_v'])

    s_cc = g['cc_g'] / np.sqrt(g['cc_v'] + eps)
    wcc_eff = g['cc_w'] * s_cc[:, None, None, None]     # [128, 256, 3, 3]
    bcc_eff = (g['cc_be'] - g['cc_m'] * s_cc).astype(f32)
    # role S convolves E_s (input channels 128:256); role Q convolves E_q
    # (0:128). Output channels are permuted so rows 0:64 are always the
    # co-half this core finalizes (S: 0:64, Q: 64:128) and rows 64:128 the
    # half it ships to its cross-role partner.
    co_perm = {'s': np.arange(128),
               'q': np.concatenate([np.arange(64, 128), np.arange(0, 64)])}
    wcc_role = {}
    bcc_role = {}
    for role, c0 in (('s', 128), ('q', 0)):
        perm = co_perm[role]
        wt = np.zeros((9, 128, 128), f32)
        for t in range(9):
            wt[t] = wcc_eff[perm][:, c0:c0 + 128, t // 3, t % 3].T
        wcc_role[role] = wt
        bcc_role[role] = bcc_eff[perm].reshape(128, 1)

    wv = np.ascontiguousarray(g['cv_w'][:, :, 0, 0].T)         # [ci, c]
    bvrow = g['cv_b'].reshape(1, 128)
    wk1 = np.ascontiguousarray(g['k1_w'][:, :, 0, 0].T)        # [ci, 64]
    wk2n = np.ascontiguousarray((-g['k2_w'][:, :, 0, 0]).T)
    wq1 = np.ascontiguousarray(g['q1_w'][:, :, 0, 0].T)
    wq2 = np.ascontiguousarray(g['q2_w'][:, :, 0, 0].T)
    w1t = np.ascontiguousarray(g['g1_w'].T) / float(N)         # [128, 8] mean-folded
    b1 = g['g1_b'].reshape(CR, 1)
    w2t = np.ascontiguousarray(g['g2_w'].T)                    # [8, 128]
    b2 = g['g2_b'].reshape(128, 1)

    def pad_img(x, h):                                          # x [256, 64, 64]
        out = np.zeros((256, NROW_IN, PW), f32)
        r0, r1 = 32 * h - 1, 32 * h + 33
        cr0, cr1 = max(r0, 0), min(r1, H)
        out[:, cr0 - r0:cr1 - r0, 1:65] = x[:, cr0:cr1, :]
        return out.reshape(2, 128, NROW_IN * PW)

    P = np.arange(128)
    def rows(sec, slot, chan_off=0):
        # AG2a table: 128-row sections [k:0, q:64]; AG2b: 256-row sections [v:0, st:128]
        if slot in ('k', 'q'):
            base = 128 * sec + (0 if slot == 'k' else 64)
        else:
            base = 256 * sec + (0 if slot == 'v' else 128)
        return (base + chan_off + P).astype(np.int32).reshape(128, 1)

    def rows64(sec_lo, slot, sec_hi):
        lo = rows(sec_lo, slot)[0:64, 0]
        hi = rows(sec_hi, slot)[0:64, 0]
        return np.concatenate([lo, hi]).astype(np.int32).reshape(128, 1)

    in_maps = []
    for c in range(8):
        b, r = c // 4, c % 4
        role_s = r < 2
        h = r % 2
        img_full = g['s'][b] if role_s else g['q'][b]
        d = {
            'img': pad_img(img_full, h),
            'wstem': ws_s if role_s else ws_q,
            'bstem': bs_s if role_s else bs_q,
            'wkp': wk1 if role_s else wk2n,
            'bkp': (g['k1_b'] if role_s else -g['k2_b']).reshape(CH, 1),
            'wqp': wq1 if role_s else wq2,
            'bqp': (g['q1_b'] if role_s else g['q2_b']).reshape(CH, 1),
            'wv': wv, 'bvrow': bvrow,
            'w1t': w1t, 'b1': b1, 'w2t': w2t, 'b2': b2,
        }
        d['wcc'] = wcc_role['s' if role_s else 'q']
        d['bcc'] = bcc_role['s' if role_s else 'q']
        # gather indices (sections: 0,1 = s-img halves; 2,3 = q-img halves).
        # u-chunk order is LOCAL-half-first: x1a covers my own pixel half so
        # PV steps 0..15 can run on the locally computed v before the v
        # exchange lands.
        slot1 = 'q' if role_s else 'k'
        d['ix_x1a'] = rows64(h, slot1, 2 + h)
        d['ix_x1b'] = rows64(1 - h, slot1, 2 + (1 - h))
        d['ix_x2'] = rows64(h, 'k' if role_s else 'q', 2 + h)
        osecs = (2, 3) if role_s else (0, 1)
        # z-partial granule of section `osec` that covers MY pixels: granule 0
        # holds osec's own half (its local-first order), granule 1 the other.
        d['ix_d1'] = ((128 * osecs[0] + P) * 2 + h).astype(np.int32).reshape(128, 1)
        d['ix_d2'] = ((128 * osecs[1] + P) * 2 + (1 - h)).astype(np.int32).reshape(128, 1)
        # partner v rows in the 2-rank ag2b table (my rank within pair = h)
        d['ix_vp'] = (128 * (1 - h) + P).astype(np.int32).reshape(128, 1)
        # halo gathers from the 2-rank halo table ([256 rows, 3 granules of
        # 64]): rank within the same-role pair = h; granule 0 = top row of
        # that core's E half, 1 = bottom row, 2 = zeros
        myrk, prk = h, 1 - h
        if h == 0:
            top = (128 * myrk + P) * 3 + 2           # zeros (image row -1)
            bot = (128 * prk + P) * 3 + 0            # partner's first row (row 32)
        else:
            top = (128 * prk + P) * 3 + 1            # partner's last row (row 31)
            bot = (128 * myrk + P) * 3 + 2           # zeros (image row 64)
        d['ix_h1'] = top.astype(np.int32).reshape(128, 1)
        d['ix_h2'] = bot.astype(np.int32).reshape(128, 1)
        # cc-partial partner = other rank of the 2-rank cross-role pair
        pprk = 1 - (0 if r < 2 else 1)
        d['ix_pp'] = (64 * pprk + np.arange(64)).astype(np.int32).reshape(64, 1)
        in_maps.append(d)
    return in_maps


def _assemble(results):
    cpam = np.zeros((B, C, H, W), np.float32)
    e_q = np.zeros((B, C, H, W), np.float32)
    e_s = np.zeros((B, C, H, W), np.float32)
    for c in range(8):
        b, r = c // 4, c % 4
        h = r % 2
        e_half = results[c]['o_e'].reshape(C, 32, W)
        tgt = e_s if r < 2 else e_q
        tgt[b, :, 32 * h:32 * h + 32, :] = e_half
        co0 = 0 if r < 2 else 64
        cpam[b, co0:co0 + 64, 32 * h:32 * h + 32, :] = \
            results[c]['o_cc'].reshape(64, 32, W)
    return cpam, e_q, e_s


def kernel(**inputs):
    nc = build_program()
    in_maps = _prep_inputs(inputs)
    res = bass_utils.run_bass_kernel_spmd(nc, in_maps, core_ids=list(range(8)))
    return _assemble(res.results)


def kernel_traced(**inputs):
    """Like kernel() but reporting a time estimate.

    Tries NTFF tracing (real HW exec time); if the profiling hook is not
    available in this environment, falls back to the Tile cost-model
    timeline simulation (single-core makespan; its collective cost model
    assumes cross-chip scale, so it substantially over-estimates the
    intra-chip AllGathers this kernel uses).
    """
    nc = build_program()
    in_maps = _prep_inputs(inputs)
    exec_ns = None
    try:
        res = bass_utils.run_bass_kernel_spmd(nc, in_maps, core_ids=list(range(8)),
                                              trace=True)
        exec_ns = res.exec_time_ns
    except Exception:
        res = bass_utils.run_bass_kernel_spmd(nc, in_maps, core_ids=list(range(8)))
    if exec_ns is None:
        try:
            from concourse.timeline_sim import TimelineSim
            exec_ns = int(TimelineSim(nc, no_exec=True, trace=False).simulate())
        except Exception:
            exec_ns = -1
    return _assemble(res.results), exec_ns

